# revision 1
# baseline (speedup 1.0000x reference)
"""Trainium2 Bass kernel for the GSAT HeteroGNN problem (8 NeuronCores).

Self-contained: hardcodes shapes/sharding; only imports the concourse
toolchain.

Strategy (dst-node sharding, SPMD over 8 cores):
  - papers split into 8 canonical chunks of 12500 (padded 12544 = 98 tiles),
    authors 8 x 6250 (padded 6272 = 49 tiles).
  - edges live on their dst's owner core, laid out host-side into 128-slot
    columns per (dst-tile, src-bank); dma_gather (int16 idx, <=32768-row
    banks) fetches source rows as [128, cols, feat] fp16.
  - segment-mean via mask-matmul: mask[e, d] = (iota==dst_local)*recip built
    with one fused tensor_scalar; TensorE accumulates aggT[feat,dst] in PSUM.
  - aggT (psum->sbuf fp16) is directly lhsT of the linear stage
    out[dst,256] = aggT.T@WlT + x_dstT.T@WrT (+ skip), roots from
    dma-transposed fp16 chunks resident in SBUF.
  - h1 (fp16) AllGather'd across cores; L2 gathers use AG-layout indices
    (identical index slabs as L1 since x tables are stored in AG layout).
  - global mean-pool via ones-column matmuls accumulating in PSUM (masked
    ones for the final partial tile); final 2-layer MLP on host in fp64.
"""
import os
import sys

try:
    import concourse  # noqa: F401
except ImportError:  # toolchain location in the grading container
    sys.path.insert(0, "/opt/trn_rl_repo")

import numpy as np
from concourse import bass, bacc, mybir, tile
from concourse import bass_utils

dt = mybir.dt

# ---------------------------------------------------------------- constants
NA, NP_, E = 50000, 100000, 300000
IN, H, OUT = 128, 256, 16
C = 8                      # cores
P = 128                    # partitions
A_CAN, P_CAN = NA // C, NP_ // C              # 6250 / 12500
A_PAD = ((A_CAN + P - 1) // P) * P            # 6272
P_PAD = ((P_CAN + P - 1) // P) * P            # 12544
NA_AG, NP_AG = C * A_PAD, C * P_PAD           # 50176 / 100352
GROUP_W = 8                # dst tiles per gather group (papers)
GROUP_B = 4                # dst tiles per gather group (authors)


def _n_banks(n_rows):
    nb = (n_rows + 32767) // 32768
    return nb, (n_rows + nb - 1) // nb


class Relation:
    """Host-side uniform structure + per-core data for one edge relation."""

    def __init__(self, src, dst, n_src, src_can, src_pad, n_dst, dst_can,
                 dst_pad, recip_dst, group):
        self.n_tiles = dst_pad // P
        self.n_banks, self.bank_rows = _n_banks(C * src_pad)
        src_ag = (src // src_can) * src_pad + (src % src_can)
        dst_core = dst // dst_can
        dst_loc = dst % dst_can
        tilei = dst_loc // P
        pos = dst_loc % P
        bank = src_ag // self.bank_rows
        src_in_bank = (src_ag % self.bank_rows).astype(np.int64)

        # per (core, tile, bank) edge lists
        T, B = self.n_tiles, self.n_banks
        key = ((dst_core * T + tilei) * B + bank).astype(np.int64)
        order = np.argsort(key, kind="stable")
        key_s = key[order]
        counts = np.bincount(key_s, minlength=C * T * B).reshape(C, T, B)
        starts = np.zeros(C * T * B + 1, np.int64)
        np.cumsum(counts.ravel(), out=starts[1:])

        # uniform column counts per (tile, bank): max over cores
        self.cols_tb = np.ceil(counts.max(axis=0) / P).astype(np.int64)  # [T,B]

        # column layout: groups of GROUP tiles; within group: bank-major,
        # then tile, then that tile's columns for the bank.
        self.groups = [list(range(g, min(g + group, T)))
                       for g in range(0, T, group)]
        col = 0
        idx_off = 0
        self.tile_cols = [[] for _ in range(T)]   # global col ids per tile
        self.col_of_tb = {}
        self.ops = []        # (bank, idx_free_off, num_idxs, col_base, n_cols)
        self.group_span = []  # (col_base, n_cols) per group
        for tiles in self.groups:
            gbase = col
            for b in range(B):
                ob = col
                for t in tiles:
                    self.col_of_tb[(t, b)] = col
                    for _ in range(int(self.cols_tb[t, b])):
                        self.tile_cols[t].append(col)
                        col += 1
                nco = col - ob
                if nco:
                    self.ops.append((b, idx_off, nco * P, ob, nco))
                    idx_off += nco * P // 16
            self.group_span.append((gbase, col - gbase))
        self.total_cols = col
        self.idx_width = idx_off
        self.max_group_cols = max(n for _, n in self.group_span) if col else 0

        # per-core slot data
        self.idx16 = np.zeros((C, P, self.idx_width), np.int16)
        self.dstl = np.full((C, P, max(col, 1)), -1.0, np.float32)
        self.recip = np.zeros((C, P, max(col, 1)), np.float32)
        src_in_bank_s = src_in_bank[order]
        pos_s = pos[order]
        recip_e_s = recip_dst[dst[order]].astype(np.float32)
        for c in range(C):
            for (b, ioff, nidx, cbase, ncols) in self.ops:
                op_idx = np.zeros(nidx, np.int16)
                # tiles covered by this op, in layout order
                j0 = 0
                for t in self._op_tiles(cbase, ncols):
                    k = (c * T + t) * B + b
                    s, e2 = starts[k], starts[k + 1]
                    m = e2 - s
                    cap = int(self.cols_tb[t, b]) * P
                    assert m <= cap
                    sl = order[s:e2]
                    op_idx[j0:j0 + m] = src_in_bank_s[s:e2]
                    # slot (p, col) for j within op: p=j%128, col=cbase+j//128
                    jj = np.arange(j0, j0 + m)
                    pp = jj % P
                    cc = cbase + jj // P
                    self.dstl[c, pp, cc] = pos_s[s:e2]
                    self.recip[c, pp, cc] = recip_e_s[s:e2]
                    j0 += cap
                # wrap int16 idx: j -> [j%16, j//16], replicate to 128 parts
                w = op_idx.reshape(-1, 16).T  # [16, nidx/16]
                self.idx16[c, :, ioff:ioff + nidx // 16] = np.tile(w, (8, 1))

    def _op_tiles(self, cbase, ncols):
        out = []
        for (t, b), c0 in self.col_of_tb.items():
            if cbase <= c0 < cbase + ncols and self.cols_tb[t, b] > 0:
                out.append((t, c0))
        return [t for t, _ in sorted(out, key=lambda x: x[1])]


def _prep(inputs):
    """All host-side preprocessing. Returns (structures, per-core in_maps,
    replicated arrays, weights for host MLP)."""
    f = lambda k: np.asarray(inputs[k], np.float32)
    x_author, x_paper = f("x_author"), f("x_paper")
    ws, wd = (np.asarray(inputs["ei_writes_src"], np.int64),
              np.asarray(inputs["ei_writes_dst"], np.int64))
    bs, bd = (np.asarray(inputs["ei_wb_src"], np.int64),
              np.asarray(inputs["ei_wb_dst"], np.int64))

    cnt_p = np.bincount(wd, minlength=NP_).astype(np.float32)
    cnt_a = np.bincount(bd, minlength=NA).astype(np.float32)
    recip_p = 1.0 / np.maximum(cnt_p, 1.0)
    recip_a = 1.0 / np.maximum(cnt_a, 1.0)

    relW = Relation(ws, wd, NA, A_CAN, A_PAD, NP_, P_CAN, P_PAD, recip_p, GROUP_W)
    relB = Relation(bs, bd, NP_, P_CAN, P_PAD, NA, A_CAN, A_PAD, recip_a, GROUP_B)

    # tables in AG layout, fp16
    xa_ag = np.zeros((NA_AG, IN), np.float16)
    xp_ag = np.zeros((NP_AG, IN), np.float16)
    for c in range(C):
        xa_ag[c * A_PAD:c * A_PAD + A_CAN] = x_author[c * A_CAN:(c + 1) * A_CAN]
        xp_ag[c * P_PAD:c * P_PAD + P_CAN] = x_paper[c * P_CAN:(c + 1) * P_CAN]

    # weight slab: 14 x [128, 256] fp16 (transposed: [in, out])
    wT = lambda k: f(k).T.astype(np.float16)       # [in, out]
    slabs = [wT("c1w_Wl"), wT("c1w_Wr"), wT("c1b_Wl"), wT("c1b_Wr")]
    for k in ("c2w_Wl", "c2w_Wr", "c2b_Wl", "c2b_Wr"):
        w2 = wT(k)                                  # [256, 256]
        slabs += [w2[:128], w2[128:]]
    slabs += [wT("skipA_W"), wT("skipP_W")]
    wslab = np.concatenate(slabs, axis=0)           # [14*128, 256]

    iota = np.broadcast_to(np.arange(P, dtype=np.float16), (P, P)).copy()
    pool_ones = np.zeros((P, 3), np.float16)
    pool_ones[:, 0] = 1.0
    pool_ones[:P_CAN - (P_PAD // P - 1) * P, 1] = 1.0   # last paper tile mask
    pool_ones[:A_CAN - (A_PAD // P - 1) * P, 2] = 1.0   # last author tile mask

    bias_nz = {k: bool(np.any(f(k))) for k in
               ("c1w_bl", "c1b_bl", "skipA_b", "skipP_b")}
    # broadcast bias tiles (only used when nonzero)
    bias_p1 = np.broadcast_to(f("c1w_bl"), (P, H)).astype(np.float32).copy()
    bias_a1 = np.broadcast_to(f("c1b_bl"), (P, H)).astype(np.float32).copy()
    bias_p2 = np.broadcast_to(f("skipP_b"), (P, H)).astype(np.float32).copy()
    bias_a2 = np.broadcast_to(f("skipA_b"), (P, H)).astype(np.float32).copy()

    in_maps = []
    for c in range(C):
        in_maps.append(dict(
            xa_tab=xa_ag, xp_tab=xp_ag,
            xa_chunk=xa_ag[c * A_PAD:(c + 1) * A_PAD],
            xp_chunk=xp_ag[c * P_PAD:(c + 1) * P_PAD],
            w_idx=relW.idx16[c], w_dstl=relW.dstl[c], w_recip=relW.recip[c],
            b_idx=relB.idx16[c], b_dstl=relB.dstl[c], b_recip=relB.recip[c],
            wslab=wslab, iota=iota, pool_ones=pool_ones,
            bias_p1=bias_p1, bias_a1=bias_a1, bias_p2=bias_p2, bias_a2=bias_a2,
        ))
    return relW, relB, in_maps, bias_nz


def _build(relW, relB, bias_nz, debug=False):
    nc = bacc.Bacc("TRN2", target_bir_lowering=False, debug=False,
                   num_devices=C)
    f16, f32, i16 = dt.float16, dt.float32, dt.int16
    ein = lambda n, s, d: nc.dram_tensor(n, s, d, kind="ExternalInput")

    xa_tab = ein("xa_tab", [NA_AG, IN], f16)
    xp_tab = ein("xp_tab", [NP_AG, IN], f16)
    xa_chunk = ein("xa_chunk", [A_PAD, IN], f16)
    xp_chunk = ein("xp_chunk", [P_PAD, IN], f16)
    w_idx = ein("w_idx", [P, relW.idx_width], i16)
    w_dstl = ein("w_dstl", [P, relW.total_cols], f32)
    w_recip = ein("w_recip", [P, relW.total_cols], f32)
    b_idx = ein("b_idx", [P, relB.idx_width], i16)
    b_dstl = ein("b_dstl", [P, relB.total_cols], f32)
    b_recip = ein("b_recip", [P, relB.total_cols], f32)
    wslab = ein("wslab", [14 * P, H], f16)
    iota_in = ein("iota", [P, P], f16)
    pool_in = ein("pool_ones", [P, 3], f16)
    bias_in = {k: ein(k, [P, H], f32)
               for k in ("bias_p1", "bias_a1", "bias_p2", "bias_a2")}

    out_pool = nc.dram_tensor("out_pool", [1, 2 * H], f32, kind="ExternalOutput")
    if debug:
        dbg_h1a = nc.dram_tensor("dbg_h1a", [A_PAD, H], f16,
                                 kind="ExternalOutput")
        dbg_h1p = nc.dram_tensor("dbg_h1p", [P_PAD, H], f16,
                                 kind="ExternalOutput")

    # weight slab order (matches _prep)
    W = {k: i for i, k in enumerate(
        ["c1w_Wl", "c1w_Wr", "c1b_Wl", "c1b_Wr",
         "c2w_Wl0", "c2w_Wl1", "c2w_Wr0", "c2w_Wr1",
         "c2b_Wl0", "c2b_Wl1", "c2b_Wr0", "c2b_Wr1",
         "skipA_W", "skipP_W"])}

    with tile.TileContext(nc) as tc:
        with tc.tile_pool(name="persist", bufs=1) as pp, \
             tc.tile_pool(name="dram", bufs=1, space="DRAM") as dp, \
             tc.tile_pool(name="work", bufs=3) as wk, \
             tc.tile_pool(name="msgs", bufs=2) as mp, \
             tc.tile_pool(name="psA", bufs=4, space="PSUM") as psA, \
             tc.tile_pool(name="psL", bufs=2, space="PSUM") as psL, \
             tc.tile_pool(name="psP", bufs=1, space="PSUM") as psP:

            # ---------------- persistent loads
            wt = pp.tile([P, 14, H], f16, name="wt", tag="wt")
            nc.sync.dma_start(out=wt[:],
                              in_=wslab[:].rearrange("(s p) d -> p s d", p=P))
            iota_t = pp.tile([P, P], f16, name="iota_t", tag="iota_t")
            nc.sync.dma_start(out=iota_t[:], in_=iota_in[:])
            pool_t = pp.tile([P, 3], f16, name="pool_t", tag="pool_t")
            nc.sync.dma_start(out=pool_t[:], in_=pool_in[:])
            meta = {}
            for nm, hnd, rel in (("w_idx", w_idx, relW), ("w_dstl", w_dstl, relW),
                                 ("w_recip", w_recip, relW),
                                 ("b_idx", b_idx, relB), ("b_dstl", b_dstl, relB),
                                 ("b_recip", b_recip, relB)):
                t = pp.tile(list(hnd.shape), hnd.dtype, name=nm + "_t")
                nc.sync.dma_start(out=t[:], in_=hnd[:])
                meta[nm] = t
            bias_t = {}
            for k, nz in (("bias_p1", bias_nz["c1w_bl"]),
                          ("bias_a1", bias_nz["c1b_bl"]),
                          ("bias_p2", bias_nz["skipP_b"]),
                          ("bias_a2", bias_nz["skipA_b"])):
                if nz:
                    t = pp.tile([P, H], f32, name=k + "_t")
                    nc.sync.dma_start(out=t[:], in_=bias_in[k][:])
                    bias_t[k] = t

            xaT = pp.tile([P, A_PAD], f16, name="xaT", tag="xaT")
            nc.sync.dma_start_transpose(out=xaT[:], in_=xa_chunk[:])
            xpT = pp.tile([P, P_PAD], f16, name="xpT", tag="xpT")
            nc.sync.dma_start_transpose(out=xpT[:], in_=xp_chunk[:])

            h1a_mine = dp.tile([A_PAD, H], f16, name="h1a_mine", tag="h1a_mine")
            h1p_mine = dp.tile([P_PAD, H], f16, name="h1p_mine", tag="h1p_mine")
            h1a_full = dp.tile([NA_AG, H], f16, name="h1a_full", tag="h1a_full")
            h1p_full = dp.tile([NP_AG, H], f16, name="h1p_full", tag="h1p_full")

            relu_f = mybir.ActivationFunctionType.Relu

            def conv(rel, table, elem, idx_t, dstl_t, recip_t, lhsWl, lhsWr,
                     rootT, skipW, skipT, h1_out, bias, pool_ps, pool_last_col):
                """One SAGE conv over `rel` for all dst tiles of this core.
                Layer-1 mode: elem=IN, h1_out set, pool_ps None.
                Layer-2 mode: elem=H, h1_out None, pool_ps set."""
                nslice = elem // P
                n_tiles = rel.n_tiles
                for gi, tiles in enumerate(rel.groups):
                    cbase, ncols = rel.group_span[gi]
                    if ncols:
                        msgs = mp.tile([P, rel.max_group_cols, elem], f16,
                                       tag="msgs" + str(elem))
                        for (b, ioff, nidx, ocb, onc) in rel.ops:
                            if not (cbase <= ocb < cbase + ncols):
                                continue
                            b0 = b * rel.bank_rows
                            b1 = min(b0 + rel.bank_rows, table.shape[0])
                            nc.gpsimd.dma_gather(
                                msgs[:, ocb - cbase:ocb - cbase + onc, :],
                                table[b0:b1, :],
                                idx_t[:, ioff:ioff + nidx // 16],
                                nidx, nidx, elem, single_packet=False)
                    for t in tiles:
                        cols = rel.tile_cols[t]
                        aggs = []
                        if cols:
                            for s in range(nslice):
                                aggs.append(psA.tile([P, P], f32, tag="agg",
                                                     name="agg",
                                                     space="PSUM"))
                            for i, cg in enumerate(cols):
                                mask = wk.tile([P, P], f16, tag="mask")
                                nc.vector.tensor_scalar(
                                    out=mask[:], in0=iota_t[:],
                                    scalar1=dstl_t[:, cg:cg + 1],
                                    scalar2=recip_t[:, cg:cg + 1],
                                    op0=mybir.AluOpType.is_equal,
                                    op1=mybir.AluOpType.mult)
                                cl = cg - cbase
                                for s in range(nslice):
                                    nc.tensor.matmul(
                                        out=aggs[s][:],
                                        lhsT=msgs[:, cl:cl + 1,
                                                  s * P:(s + 1) * P],
                                        rhs=mask[:],
                                        start=(i == 0), stop=(i == len(cols) - 1))
                            aggT = []
                            for s in range(nslice):
                                a = wk.tile([P, P], f16, tag="aggT")
                                nc.scalar.copy(out=a[:], in_=aggs[s][:])
                                aggT.append(a)
                        lin = psL.tile([P, H], f32, tag="lin", space="PSUM")
                        first = True
                        if cols:
                            for s in range(nslice):
                                nc.tensor.matmul(
                                    out=lin[:], lhsT=aggT[s][:],
                                    rhs=wt[:, lhsWl[s]:lhsWl[s] + 1, :],
                                    start=first, stop=False)
                                first = False
                        for s in range(nslice):
                            nc.tensor.matmul(
                                out=lin[:],
                                lhsT=rootT[s][:, t * P:(t + 1) * P],
                                rhs=wt[:, lhsWr[s]:lhsWr[s] + 1, :],
                                start=first,
                                stop=(skipW is None and s == nslice - 1))
                            first = False
                        if skipW is not None:
                            nc.tensor.matmul(out=lin[:],
                                             lhsT=skipT[:, t * P:(t + 1) * P],
                                             rhs=wt[:, skipW:skipW + 1, :],
                                             start=False, stop=True)
                        h = wk.tile([P, H], f16, tag="relu")
                        if bias is None:
                            nc.scalar.activation(out=h[:], in_=lin[:],
                                                 func=relu_f)
                        else:
                            tmp = wk.tile([P, H], f32, tag="btmp")
                            nc.vector.tensor_add(out=tmp[:], in0=lin[:],
                                                 in1=bias[:])
                            nc.scalar.activation(out=h[:], in_=tmp[:],
                                                 func=relu_f)
                        if h1_out is not None:
                            nc.sync.dma_start(out=h1_out[t * P:(t + 1) * P, :],
                                              in_=h[:])
                        if pool_ps is not None:
                            oc = pool_last_col if t == n_tiles - 1 else 0
                            nc.tensor.matmul(
                                out=pool_ps[:], lhsT=pool_t[:, oc:oc + 1],
                                rhs=h[:], start=(t == 0),
                                stop=(t == n_tiles - 1), skip_group_check=True)

            rg = [list(range(C))]

            # -------- layer 1: authors (wb relation: src papers -> dst authors)
            conv(relB, xp_tab, IN, meta["b_idx"], meta["b_dstl"],
                 meta["b_recip"], [W["c1b_Wl"]], [W["c1b_Wr"]], [xaT], None,
                 None, h1a_mine, bias_t.get("bias_a1"), None, 0)
            nc.gpsimd.collective_compute(
                "AllGather", mybir.AluOpType.bypass, replica_groups=rg,
                ins=[h1a_mine.opt()], outs=[h1a_full.opt()])

            # -------- layer 1: papers (writes relation)
            conv(relW, xa_tab, IN, meta["w_idx"], meta["w_dstl"],
                 meta["w_recip"], [W["c1w_Wl"]], [W["c1w_Wr"]], [xpT], None,
                 None, h1p_mine, bias_t.get("bias_p1"), None, 0)
            nc.gpsimd.collective_compute(
                "AllGather", mybir.AluOpType.bypass, replica_groups=rg,
                ins=[h1p_mine.opt()], outs=[h1p_full.opt()])

            # -------- transposed local h1 chunks for the L2 root terms
            h1aT = []
            for s in range(2):
                t = pp.tile([P, A_PAD], f16, name=f"h1aT{s}", tag=f"h1aT{s}")
                nc.sync.dma_start_transpose(
                    out=t[:], in_=h1a_mine[:, s * P:(s + 1) * P])
                h1aT.append(t)
            h1pT = []
            for s in range(2):
                t = pp.tile([P, P_PAD], f16, name=f"h1pT{s}", tag=f"h1pT{s}")
                nc.sync.dma_start_transpose(
                    out=t[:], in_=h1p_mine[:, s * P:(s + 1) * P])
                h1pT.append(t)

            # -------- layer 2 + skip + pool
            pool_p = psP.tile([1, H], f32, name="pool_p", tag="pool_p", space="PSUM")
            pool_a = psP.tile([1, H], f32, name="pool_a", tag="pool_a", space="PSUM")
            conv(relW, h1a_full, H, meta["w_idx"], meta["w_dstl"],
                 meta["w_recip"], [W["c2w_Wl0"], W["c2w_Wl1"]],
                 [W["c2w_Wr0"], W["c2w_Wr1"]], h1pT, W["skipP_W"], xpT,
                 None, bias_t.get("bias_p2"), pool_p, 1)
            conv(relB, h1p_full, H, meta["b_idx"], meta["b_dstl"],
                 meta["b_recip"], [W["c2b_Wl0"], W["c2b_Wl1"]],
                 [W["c2b_Wr0"], W["c2b_Wr1"]], h1aT, W["skipA_W"], xaT,
                 None, bias_t.get("bias_a2"), pool_a, 2)

            pool_sb = wk.tile([1, 2 * H], f32, tag="poolout")
            nc.vector.tensor_copy(out=pool_sb[:, 0:H], in_=pool_a[:])
            nc.vector.tensor_copy(out=pool_sb[:, H:2 * H], in_=pool_p[:])
            nc.sync.dma_start(out=out_pool[:], in_=pool_sb[:])

            if debug:
                nc.sync.dma_start(out=dbg_h1a[:], in_=h1a_mine[:])
                nc.sync.dma_start(out=dbg_h1p[:], in_=h1p_mine[:])

    nc.compile()
    return nc


def kernel(**inputs):
    debug = bool(int(os.environ.get("GNN_DEBUG", "0")))
    trace = bool(int(os.environ.get("GNN_TRACE", "0")))
    relW, relB, in_maps, bias_nz = _prep(inputs)
    nc = _build(relW, relB, bias_nz, debug=debug)
    res = bass_utils.run_bass_kernel_spmd(
        nc, in_maps, core_ids=list(range(C)), trace=trace)
    kernel.last_results = res

    pools = np.stack([res.results[c]["out_pool"] for c in range(C)])  # [C,1,2H]
    sum_a = pools[:, 0, :H].astype(np.float64).sum(axis=0)
    sum_p = pools[:, 0, H:].astype(np.float64).sum(axis=0)
    pooled = np.concatenate([sum_a / NA, sum_p / NP_])[None, :]  # [1, 2H]
    W1 = np.asarray(inputs["cls_W1"], np.float64)
    b1 = np.asarray(inputs["cls_b1"], np.float64)
    W2 = np.asarray(inputs["cls_W2"], np.float64)
    b2 = np.asarray(inputs["cls_b2"], np.float64)
    h = np.maximum(pooled @ W1.T + b1, 0.0)
    out = h @ W2.T + b2
    return out.astype(np.float32)



# revision 17
# speedup vs baseline: 1.3569x; 1.3569x over previous
"""Trainium2 Bass kernel for the GSAT HeteroGNN problem (8 NeuronCores).

Self-contained: hardcodes shapes/sharding; only imports the concourse
toolchain.

Strategy (dst-node sharding, SPMD over 8 cores):
  - papers split into 8 canonical chunks of 12500 (padded 12544 = 98 tiles),
    authors 8 x 6250 (padded 6272 = 49 tiles).
  - edges live on their dst's owner core, laid out host-side into 128-slot
    columns per (4-tile window, src-bank); dma_gather (int16 idx) fetches
    fp8 source rows as [128, cols, feat].
  - segment-mean via host-precomputed fp8 masks streamed by DMA:
    mask[slot, dst_in_window] = 1/deg(dst); TensorE accumulates
    aggT[feat, 512] in PSUM per window (no on-device mask building).
  - L1 gathers read per-core COMPACT fp8 tables (only the <=32k rows this
    core references -> single bank, minimal column padding).
  - L1 outputs h1 are written twice: fp8 rows into a local chunk that a
    Shared-output AllGather assembles into a shared fp8 table (each rank
    contributes only its 1.6-3.2MB shard; the old Local-output AllGathers
    moved 77MB/core), and fp16 into a local chunk used for DMA transposes
    (L2 root terms).
  - L2 gathers read the shared fp8 h1 tables directly.
  - all DMA transposes are placed before any collective in program order
    (the scheduler serializes transposes with collectives).
  - global mean-pool via ones-column matmuls accumulating in PSUM; final
    2-layer MLP on host in fp64.
"""
import os
import sys

try:
    import concourse  # noqa: F401
except ImportError:  # toolchain location in the grading container
    sys.path.insert(0, "/opt/trn_rl_repo")

import numpy as np
import ml_dtypes
from concourse import bass, bacc, mybir, tile  # noqa: F401
from concourse import bass_utils

dt = mybir.dt
F8 = ml_dtypes.float8_e4m3

# ---------------------------------------------------------------- constants
NA, NP_, E = 50000, 100000, 300000
IN, H, OUT = 128, 256, 16
C = 8                      # cores
P = 128                    # partitions
A_CAN, P_CAN = NA // C, NP_ // C              # 6250 / 12500
A_PAD = ((A_CAN + P - 1) // P) * P            # 6272
P_PAD = ((P_CAN + P - 1) // P) * P            # 12544
NA_AG, NP_AG = C * A_PAD, C * P_PAD           # 50176 / 100352
WIN = 4                    # dst tiles per PSUM window (512 dsts)
WD = WIN * P               # window width in dsts


class RelLayer:
    """Host-side layout for one (relation, layer): slot columns per
    (window, bank), uniform across cores (max-over-cores column counts),
    int16 gather indices and fp8 recip masks."""

    def __init__(self, row_of, dst_owner, dstl, n_dst_can, n_dst_pad,
                 recip_dst_local, table_rows):
        # row_of: [C] list of per-edge row ids (into this layer's table)
        # dst_owner/dstl: per-edge owner core and local dst id (global arrays
        # already split: row_of[c] aligned with dstl[c])
        self.n_tiles = n_dst_pad // P
        self.n_win = (self.n_tiles + WIN - 1) // WIN
        nb = (table_rows + 32767) // 32768
        self.n_banks = nb
        self.bank_rows = (table_rows + nb - 1) // nb
        self.table_rows = table_rows

        # per-core per-cell counts -> uniform ncols
        ncols = np.zeros((self.n_win, nb), np.int64)
        per_core = []
        for c in range(C):
            rows, dl = row_of[c], dstl[c]
            w = dl // WD
            b = rows // self.bank_rows
            cnt = np.zeros((self.n_win, nb), np.int64)
            np.add.at(cnt, (w, b), 1)
            ncols = np.maximum(ncols, (cnt + P - 1) // P)
            per_core.append((rows, dl, w, b))
        self.ncols = ncols

        # global column layout: window-major, bank-minor
        self.col_base = np.zeros(self.n_win + 1, np.int64)
        self.ops = []              # per window: list of (bank, ioff, nidx, lcb)
        ioff = 0
        col = 0
        for w in range(self.n_win):
            self.col_base[w] = col
            wops = []
            lcb = 0
            for b in range(nb):
                nco = int(ncols[w, b])
                if nco:
                    wops.append((b, ioff, nco * P, lcb))
                    ioff += nco * P // 16
                    lcb += nco
                    col += nco
            self.ops.append(wops)
        self.col_base[self.n_win] = col
        self.total_cols = col
        self.idx_width = ioff
        self.wcols = np.diff(self.col_base).astype(np.int64)
        self.max_wcols = int(self.wcols.max()) if col else 0
        self.total_idx = col * P

        # per-core idx + masks
        self.idx16 = np.zeros((C, P, max(self.idx_width, 1)), np.int16)
        self.masks = np.zeros((C, P, max(col, 1), WD), F8)
        cell_base = {}
        lcb_map = {}
        for w in range(self.n_win):
            for (b, io, nidx, lcb) in self.ops[w]:
                cell_base[(w, b)] = io
                lcb_map[(w, b)] = self.col_base[w] + lcb
        for c in range(C):
            rows, dl, w_e, b_e = per_core[c]
            order = np.argsort(w_e * nb + b_e, kind="stable")
            rows_s, dl_s, w_s, b_s = rows[order], dl[order], w_e[order], b_e[order]
            rec_s = recip_dst_local[c][dl_s].astype(np.float32)
            rib_s = (rows_s % self.bank_rows).astype(np.int64)
            # rank within each (w, b) run
            key = w_s * nb + b_s
            # j = index within cell
            cellcnt = np.bincount(key, minlength=self.n_win * nb)
            starts = np.zeros(self.n_win * nb + 1, np.int64)
            np.cumsum(cellcnt, out=starts[1:])
            j = np.arange(len(key)) - starts[key]
            # idx slab (flat over ops)
            flat = np.zeros(max(self.idx_width, 1) * 16, np.int16)
            iobase = np.array([cell_base.get((w, b), -1) * 16
                               for w in range(self.n_win) for b in range(nb)]
                              ).reshape(self.n_win, nb)
            pos = iobase[w_s, b_s] + j
            flat[pos] = rib_s.astype(np.int16)
            w16 = flat.reshape(-1, 16).T       # [16, width]
            self.idx16[c] = np.tile(w16, (8, 1))
            # masks
            gcol = np.array([lcb_map.get((w, b), 0)
                             for w in range(self.n_win) for b in range(nb)]
                            ).reshape(self.n_win, nb)
            cc = gcol[w_s, b_s] + j // P
            pp = j % P
            off = dl_s - w_s * WD
            self.masks[c][pp, cc, off] = rec_s.astype(F8)


def _prep(inputs):
    f = lambda k: np.asarray(inputs[k], np.float32)
    x_author, x_paper = f("x_author"), f("x_paper")
    ws, wd = (np.asarray(inputs["ei_writes_src"], np.int64),
              np.asarray(inputs["ei_writes_dst"], np.int64))
    bs, bd = (np.asarray(inputs["ei_wb_src"], np.int64),
              np.asarray(inputs["ei_wb_dst"], np.int64))

    cnt_p = np.bincount(wd, minlength=NP_).astype(np.float32)
    cnt_a = np.bincount(bd, minlength=NA).astype(np.float32)
    recip_p = 1.0 / np.maximum(cnt_p, 1.0)
    recip_a = 1.0 / np.maximum(cnt_a, 1.0)

    # split edges by dst owner
    def split(src, dst, dst_can):
        srcs, dstls = [], []
        for c in range(C):
            m = (dst // dst_can) == c
            srcs.append(src[m])
            dstls.append((dst[m] % dst_can).astype(np.int64))
        return srcs, dstls

    w_src, w_dstl = split(ws, wd, P_CAN)     # writes: dst papers
    b_src, b_dstl = split(bs, bd, A_CAN)     # wb: dst authors

    # L1 compact tables (per-core unique srcs)
    uniqW = [np.unique(s) for s in w_src]    # authors referenced per core
    uniqB = [np.unique(s) for s in b_src]    # papers referenced per core
    rowsW = ((max(len(u) for u in uniqW) + P - 1) // P) * P
    rowsB = ((max(len(u) for u in uniqB) + P - 1) // P) * P
    assert rowsW <= 32768 and rowsB <= 32768
    xa_cmp = np.zeros((C, rowsW, IN), np.float16)
    xp_cmp = np.zeros((C, rowsB, IN), np.float16)
    for c in range(C):
        xa_cmp[c, :len(uniqW[c])] = x_author[uniqW[c]].astype(np.float16)
        xp_cmp[c, :len(uniqB[c])] = x_paper[uniqB[c]].astype(np.float16)

    recip_p_loc = [recip_p[c * P_CAN:(c + 1) * P_CAN] for c in range(C)]
    recip_a_loc = [recip_a[c * A_CAN:(c + 1) * A_CAN] for c in range(C)]

    # AG row mapping for L2 tables
    agW = [(s // A_CAN) * A_PAD + (s % A_CAN) for s in w_src]
    agB = [(s // P_CAN) * P_PAD + (s % P_CAN) for s in b_src]
    cmpW = [np.searchsorted(uniqW[c], w_src[c]) for c in range(C)]
    cmpB = [np.searchsorted(uniqB[c], b_src[c]) for c in range(C)]

    rels = dict(
        W1=RelLayer(cmpW, None, w_dstl, P_CAN, P_PAD, recip_p_loc, rowsW),
        B1=RelLayer(cmpB, None, b_dstl, A_CAN, A_PAD, recip_a_loc, rowsB),
        W2=RelLayer(agW, None, w_dstl, P_CAN, P_PAD, recip_p_loc, NA_AG),
        B2=RelLayer(agB, None, b_dstl, A_CAN, A_PAD, recip_a_loc, NP_AG),
    )

    # fp16 local chunks (root/skip transposes)
    xa_chunk = np.zeros((C, A_PAD, IN), np.float16)
    xp_chunk = np.zeros((C, P_PAD, IN), np.float16)
    for c in range(C):
        xa_chunk[c, :A_CAN] = x_author[c * A_CAN:(c + 1) * A_CAN]
        xp_chunk[c, :P_CAN] = x_paper[c * P_CAN:(c + 1) * P_CAN]

    # weight slab: 14 x [128, 256] fp16 (transposed: [in, out])
    wT = lambda k: f(k).T.astype(np.float16)
    slabs = [wT("c1w_Wl"), wT("c1w_Wr"), wT("c1b_Wl"), wT("c1b_Wr")]
    for k in ("c2w_Wl", "c2w_Wr", "c2b_Wl", "c2b_Wr"):
        w2 = wT(k)
        slabs += [w2[:128], w2[128:]]
    slabs += [wT("skipA_W"), wT("skipP_W")]
    wslab = np.concatenate(slabs, axis=0)          # [14*128, 256]

    pool_ones = np.zeros((P, 3), np.float16)
    pool_ones[:, 0] = 1.0
    pool_ones[:P_CAN - (P_PAD // P - 1) * P, 1] = 1.0   # last paper tile mask
    pool_ones[:A_CAN - (A_PAD // P - 1) * P, 2] = 1.0   # last author tile mask

    bias_nz = {k: bool(np.any(f(k))) for k in
               ("c1w_bl", "c1b_bl", "skipA_b", "skipP_b")}
    bias_arr = {k: np.broadcast_to(f(k2), (P, H)).astype(np.float32).copy()
                for k, k2 in (("bias_p1", "c1w_bl"), ("bias_a1", "c1b_bl"),
                              ("bias_p2", "skipP_b"), ("bias_a2", "skipA_b"))}

    in_maps = []
    for c in range(C):
        m = dict(
            xa_cmp=xa_cmp[c], xp_cmp=xp_cmp[c],
            xa_chunk=xa_chunk[c], xp_chunk=xp_chunk[c],
            wslab=wslab, pool_ones=pool_ones,
        )
        for nm, rl in rels.items():
            m["idx_" + nm] = rl.idx16[c]
            m["mask_" + nm] = rl.masks[c].reshape(P, -1)
        for k, arr in bias_arr.items():
            m[k] = arr
        in_maps.append(m)
    return rels, in_maps, bias_nz


def _build(rels, bias_nz, debug=False):
    nc = bacc.Bacc("TRN2", target_bir_lowering=False, debug=False,
                   num_devices=C)
    f16, f32, i16, f8 = dt.float16, dt.float32, dt.int16, dt.float8e4
    ein = lambda n, s, d: nc.dram_tensor(n, s, d, kind="ExternalInput")

    xa_cmp = ein("xa_cmp", [rels["W1"].table_rows, IN], f16)
    xp_cmp = ein("xp_cmp", [rels["B1"].table_rows, IN], f16)
    xa_chunk = ein("xa_chunk", [A_PAD, IN], f16)
    xp_chunk = ein("xp_chunk", [P_PAD, IN], f16)
    wslab = ein("wslab", [14 * P, H], f16)
    pool_in = ein("pool_ones", [P, 3], f16)
    idx_h, mask_h = {}, {}
    for nm, rl in rels.items():
        idx_h[nm] = ein("idx_" + nm, [P, max(rl.idx_width, 1)], i16)
        mask_h[nm] = ein("mask_" + nm, [P, max(rl.total_cols, 1) * WD], f8)
    bias_in = {k: ein(k, [P, H], f32)
               for k in ("bias_p1", "bias_a1", "bias_p2", "bias_a2")}

    out_pool = nc.dram_tensor("out_pool", [1, 2 * H], f32,
                              kind="ExternalOutput")
    if debug:
        dbg_h1a = nc.dram_tensor("dbg_h1a", [A_PAD, H], f16,
                                 kind="ExternalOutput")
        dbg_h1p = nc.dram_tensor("dbg_h1p", [P_PAD, H], f16,
                                 kind="ExternalOutput")

    W = {k: i for i, k in enumerate(
        ["c1w_Wl", "c1w_Wr", "c1b_Wl", "c1b_Wr",
         "c2w_Wl0", "c2w_Wl1", "c2w_Wr0", "c2w_Wr1",
         "c2b_Wl0", "c2b_Wl1", "c2b_Wr0", "c2b_Wr1",
         "skipA_W", "skipP_W"])}
    relu_f = mybir.ActivationFunctionType.Relu
    rg = [list(range(C))]
    MAXW = max(rl.max_wcols for rl in rels.values())

    with tile.TileContext(nc) as tc:
        with tc.tile_pool(name="persist", bufs=1) as pp, \
             tc.tile_pool(name="dram", bufs=1, space="DRAM") as dp, \
             tc.tile_pool(name="work", bufs=3) as wk, \
             tc.tile_pool(name="msgs", bufs=2) as mp, \
             tc.tile_pool(name="maskp", bufs=2) as mk, \
             tc.tile_pool(name="psA", bufs=4, space="PSUM") as psA, \
             tc.tile_pool(name="psL", bufs=2, space="PSUM") as psL, \
             tc.tile_pool(name="psP", bufs=1, space="PSUM") as psP:

            # ---------------- persistent loads
            wt = pp.tile([P, 14, H], f16, name="wt", tag="wt")
            nc.sync.dma_start(out=wt[:],
                              in_=wslab[:].rearrange("(s p) d -> p s d", p=P))
            pool_t = pp.tile([P, 3], f16, name="pool_t", tag="pool_t")
            nc.sync.dma_start(out=pool_t[:], in_=pool_in[:])
            idx_t = {}
            for nm, rl in rels.items():
                t = pp.tile([P, max(rl.idx_width, 1)], i16, name="idx" + nm)
                nc.sync.dma_start(out=t[:], in_=idx_h[nm][:])
                idx_t[nm] = t
            bias_t = {}
            for k, nz in (("bias_p1", bias_nz["c1w_bl"]),
                          ("bias_a1", bias_nz["c1b_bl"]),
                          ("bias_p2", bias_nz["skipP_b"]),
                          ("bias_a2", bias_nz["skipA_b"])):
                if nz:
                    t = pp.tile([P, H], f32, name=k + "_t")
                    nc.sync.dma_start(out=t[:], in_=bias_in[k][:])
                    bias_t[k] = t

            xaT = pp.tile([P, A_PAD], f16, name="xaT", tag="xaT")
            nc.sync.dma_start_transpose(out=xaT[:], in_=xa_chunk[:])
            xpT = pp.tile([P, P_PAD], f16, name="xpT", tag="xpT")
            nc.sync.dma_start_transpose(out=xpT[:], in_=xp_chunk[:])

            # h1 tables: fp8 local shard -> Shared-output AllGather table
            # (fp16 local shard feeds the DMA transposes for L2 root terms)
            h1a_sh = dp.tile([NA_AG, H], f8, name="h1a_sh", tag="h1a_sh",
                             addr_space="Shared")
            h1p_sh = dp.tile([NP_AG, H], f8, name="h1p_sh", tag="h1p_sh",
                             addr_space="Shared")
            h1a_l8 = dp.tile([A_PAD, H], f8, name="h1a_l8", tag="h1a_l8")
            h1p_l8 = dp.tile([P_PAD, H], f8, name="h1p_l8", tag="h1p_l8")
            h1a_loc = dp.tile([A_PAD, H], f16, name="h1a_loc", tag="h1a_loc")
            h1p_loc = dp.tile([P_PAD, H], f16, name="h1p_loc", tag="h1p_loc")

            def conv(nm, table, elem, Wl, Wr, rootT, skipW, skipT, bias,
                     h_l8, h_loc, pool_ps, pool_last_col):
                rl = rels[nm]
                nslice = elem // P
                it = idx_t[nm]
                mdt = f16 if nslice == 1 else f8
                for w in range(rl.n_win):
                    wc = int(rl.wcols[w])
                    cb = int(rl.col_base[w])
                    aggT = []
                    if wc:
                        msgs = mp.tile([P, MAXW, elem], mdt,
                                       tag=f"msgs{nslice}")
                        for (b, ioff, nidx, lcb) in rl.ops[w]:
                            b0 = b * rl.bank_rows
                            b1 = min(b0 + rl.bank_rows, rl.table_rows)
                            nc.gpsimd.dma_gather(
                                msgs[:, lcb:lcb + nidx // P, :elem],
                                table[b0:b1, :],
                                it[:, ioff:ioff + nidx // 16],
                                nidx, nidx, elem, single_packet=False)
                        mask_t = mk.tile([P, MAXW * WD], f8, tag="mask")
                        nc.sync.dma_start(
                            out=mask_t[:, :wc * WD],
                            in_=mask_h[nm][:, cb * WD:(cb + wc) * WD])
                        aggs = []
                        for s in range(nslice):
                            aggs.append(psA.tile([P, WD], f32, tag="agg",
                                                 name="agg", space="PSUM"))
                        for i in range(wc):
                            for s in range(nslice):
                                nc.tensor.matmul(
                                    out=aggs[s][:],
                                    lhsT=msgs[:, i:i + 1, s * P:(s + 1) * P],
                                    rhs=mask_t[:, i * WD:(i + 1) * WD],
                                    start=(i == 0), stop=(i == wc - 1))
                        for s in range(nslice):
                            a = wk.tile([P, WD], f16, tag="aggT")
                            nc.scalar.copy(out=a[:], in_=aggs[s][:])
                            aggT.append(a)
                    for tl in range(min(WIN, rl.n_tiles - w * WIN)):
                        t = w * WIN + tl
                        lin = psL.tile([P, H], f32, tag="lin", space="PSUM")
                        first = True
                        if wc:
                            for s in range(nslice):
                                nc.tensor.matmul(
                                    out=lin[:],
                                    lhsT=aggT[s][:, tl * P:(tl + 1) * P],
                                    rhs=wt[:, Wl[s]:Wl[s] + 1, :],
                                    start=first, stop=False)
                                first = False
                        for s in range(nslice):
                            nc.tensor.matmul(
                                out=lin[:],
                                lhsT=rootT[s][:, t * P:(t + 1) * P],
                                rhs=wt[:, Wr[s]:Wr[s] + 1, :],
                                start=first,
                                stop=(skipW is None and s == nslice - 1))
                            first = False
                        if skipW is not None:
                            nc.tensor.matmul(
                                out=lin[:], lhsT=skipT[:, t * P:(t + 1) * P],
                                rhs=wt[:, skipW:skipW + 1, :],
                                start=False, stop=True)
                        h16 = wk.tile([P, H], f16, tag="h16")
                        if bias is None:
                            nc.scalar.activation(out=h16[:], in_=lin[:],
                                                 func=relu_f)
                        else:
                            tmp = wk.tile([P, H], f32, tag="btmp")
                            nc.vector.tensor_add(out=tmp[:], in0=lin[:],
                                                 in1=bias[:])
                            nc.scalar.activation(out=h16[:], in_=tmp[:],
                                                 func=relu_f)
                        if h_l8 is not None:
                            h8 = wk.tile([P, H], f8, tag="h8")
                            nc.vector.tensor_copy(out=h8[:], in_=h16[:])
                            nc.scalar.dma_start(
                                out=h_l8[t * P:(t + 1) * P, :], in_=h8[:])
                            nc.scalar.dma_start(
                                out=h_loc[t * P:(t + 1) * P, :], in_=h16[:])
                        if pool_ps is not None:
                            oc = pool_last_col if t == rl.n_tiles - 1 else 0
                            nc.tensor.matmul(
                                out=pool_ps[:], lhsT=pool_t[:, oc:oc + 1],
                                rhs=h16[:], start=(t == 0),
                                stop=(t == rl.n_tiles - 1),
                                skip_group_check=True)

            # -------- layer 1: authors (wb: src papers -> dst authors)
            conv("B1", xp_cmp, IN, [W["c1b_Wl"]], [W["c1b_Wr"]], [xaT],
                 None, None, bias_t.get("bias_a1"), h1a_l8, h1a_loc, None, 0)
            nc.gpsimd.collective_compute(
                "AllGather", mybir.AluOpType.bypass, replica_groups=rg,
                ins=[h1a_l8.opt()], outs=[h1a_sh.opt()])

            # -------- layer 1: papers (writes: src authors -> dst papers)
            conv("W1", xa_cmp, IN, [W["c1w_Wl"]], [W["c1w_Wr"]], [xpT],
                 None, None, bias_t.get("bias_p1"), h1p_l8, h1p_loc, None, 0)

            # -------- transposed local h1 chunks (before the 2nd collective)
            h1aT = []
            for s in range(2):
                t = pp.tile([P, A_PAD], f16, name=f"h1aT{s}", tag=f"h1aT{s}")
                nc.sync.dma_start_transpose(
                    out=t[:], in_=h1a_loc[:, s * P:(s + 1) * P])
                h1aT.append(t)
            h1pT = []
            for s in range(2):
                t = pp.tile([P, P_PAD], f16, name=f"h1pT{s}", tag=f"h1pT{s}")
                nc.sync.dma_start_transpose(
                    out=t[:], in_=h1p_loc[:, s * P:(s + 1) * P])
                h1pT.append(t)

            nc.gpsimd.collective_compute(
                "AllGather", mybir.AluOpType.bypass, replica_groups=rg,
                ins=[h1p_l8.opt()], outs=[h1p_sh.opt()])

            # -------- layer 2: papers (gathers h1a from shared table)
            pool_p = psP.tile([1, H], f32, name="pool_p", tag="pool_p",
                              space="PSUM")
            pool_a = psP.tile([1, H], f32, name="pool_a", tag="pool_a",
                              space="PSUM")
            conv("W2", h1a_sh, H, [W["c2w_Wl0"], W["c2w_Wl1"]],
                 [W["c2w_Wr0"], W["c2w_Wr1"]], h1pT, W["skipP_W"], xpT,
                 bias_t.get("bias_p2"), None, None, pool_p, 1)

            # -------- layer 2: authors
            conv("B2", h1p_sh, H, [W["c2b_Wl0"], W["c2b_Wl1"]],
                 [W["c2b_Wr0"], W["c2b_Wr1"]], h1aT, W["skipA_W"], xaT,
                 bias_t.get("bias_a2"), None, None, pool_a, 2)

            pool_sb = wk.tile([1, 2 * H], f32, tag="poolout")
            nc.vector.tensor_copy(out=pool_sb[:, 0:H], in_=pool_a[:])
            nc.vector.tensor_copy(out=pool_sb[:, H:2 * H], in_=pool_p[:])
            nc.sync.dma_start(out=out_pool[:], in_=pool_sb[:])

            if debug:
                nc.sync.dma_start(out=dbg_h1a[:], in_=h1a_loc[:])
                nc.sync.dma_start(out=dbg_h1p[:], in_=h1p_loc[:])

    nc.compile()
    return nc


def kernel(**inputs):
    debug = bool(int(os.environ.get("GNN_DEBUG", "0")))
    trace = bool(int(os.environ.get("GNN_TRACE", "0")))
    rels, in_maps, bias_nz = _prep(inputs)
    nc = _build(rels, bias_nz, debug=debug)
    res = bass_utils.run_bass_kernel_spmd(
        nc, in_maps, core_ids=list(range(C)), trace=trace)
    kernel.last_results = res

    pools = np.stack([res.results[c]["out_pool"] for c in range(C)])
    sum_a = pools[:, 0, :H].astype(np.float64).sum(axis=0)
    sum_p = pools[:, 0, H:].astype(np.float64).sum(axis=0)
    pooled = np.concatenate([sum_a / NA, sum_p / NP_])[None, :]
    W1 = np.asarray(inputs["cls_W1"], np.float64)
    b1 = np.asarray(inputs["cls_b1"], np.float64)
    W2 = np.asarray(inputs["cls_W2"], np.float64)
    b2 = np.asarray(inputs["cls_b2"], np.float64)
    h = np.maximum(pooled @ W1.T + b1, 0.0)
    out = h @ W2.T + b2
    return out.astype(np.float32)


# revision 22
# speedup vs baseline: 1.4147x; 1.0425x over previous
"""Trainium2 Bass kernel for the GSAT HeteroGNN problem (8 NeuronCores).

Self-contained: hardcodes shapes/sharding; only imports the concourse
toolchain.

Strategy (dst-node sharding, SPMD over 8 cores):
  - papers split into 8 canonical chunks of 12500 (padded 12544 = 98 tiles),
    authors 8 x 6250 (padded 6272 = 49 tiles).
  - edges live on their dst's owner core, laid out host-side into 128-slot
    columns per (4-tile window, src-bank); dma_gather (int16 idx) fetches
    fp8 source rows as [128, cols, feat].
  - segment-mean via host-precomputed fp8 masks streamed by DMA:
    mask[slot, dst_in_window] = 1/deg(dst); TensorE accumulates
    aggT[feat, 512] in PSUM per window (no on-device mask building).
  - L1 gathers read per-core COMPACT fp8 tables (only the <=32k rows this
    core references -> single bank, minimal column padding).
  - L1 outputs h1 are written twice: fp8 rows into a local chunk that a
    Shared-output AllGather assembles into a shared fp8 table (each rank
    contributes only its 1.6-3.2MB shard; the old Local-output AllGathers
    moved 77MB/core), and fp16 into a local chunk used for DMA transposes
    (L2 root terms).
  - L2 gathers read the shared fp8 h1 tables directly.
  - all DMA transposes are placed before any collective in program order
    (the scheduler serializes transposes with collectives).
  - global mean-pool via ones-column matmuls accumulating in PSUM; final
    2-layer MLP on host in fp64.
"""
import os
import sys

try:
    import concourse  # noqa: F401
except ImportError:  # toolchain location in the grading container
    sys.path.insert(0, "/opt/trn_rl_repo")

import numpy as np
import ml_dtypes
from concourse import bass, bacc, mybir, tile  # noqa: F401
from concourse import bass_utils

dt = mybir.dt
F8 = ml_dtypes.float8_e4m3

# ---------------------------------------------------------------- constants
NA, NP_, E = 50000, 100000, 300000
IN, H, OUT = 128, 256, 16
C = 8                      # cores
P = 128                    # partitions
A_CAN, P_CAN = NA // C, NP_ // C              # 6250 / 12500
A_PAD = ((A_CAN + P - 1) // P) * P            # 6272
P_PAD = ((P_CAN + P - 1) // P) * P            # 12544
NA_AG, NP_AG = C * A_PAD, C * P_PAD           # 50176 / 100352
WIN = 4                    # dst tiles per PSUM window (512 dsts)
WD = WIN * P               # window width in dsts


class RelLayer:
    """Host-side layout for one (relation, layer): slot columns per
    (window, bank), uniform across cores (max-over-cores column counts),
    int16 gather indices and fp8 recip masks."""

    def __init__(self, row_of, dst_owner, dstl, n_dst_can, n_dst_pad,
                 recip_dst_local, table_rows):
        # row_of: [C] list of per-edge row ids (into this layer's table)
        # dst_owner/dstl: per-edge owner core and local dst id (global arrays
        # already split: row_of[c] aligned with dstl[c])
        self.n_tiles = n_dst_pad // P
        self.n_win = (self.n_tiles + WIN - 1) // WIN
        nb = (table_rows + 32767) // 32768
        self.n_banks = nb
        self.bank_rows = (table_rows + nb - 1) // nb
        self.table_rows = table_rows

        # per-core per-cell counts -> uniform ncols
        ncols = np.zeros((self.n_win, nb), np.int64)
        per_core = []
        for c in range(C):
            rows, dl = row_of[c], dstl[c]
            w = dl // WD
            b = rows // self.bank_rows
            cnt = np.zeros((self.n_win, nb), np.int64)
            np.add.at(cnt, (w, b), 1)
            ncols = np.maximum(ncols, (cnt + P - 1) // P)
            per_core.append((rows, dl, w, b))
        self.ncols = ncols

        # global column layout: window-major, bank-minor
        self.col_base = np.zeros(self.n_win + 1, np.int64)
        self.ops = []              # per window: list of (bank, ioff, nidx, lcb)
        ioff = 0
        col = 0
        for w in range(self.n_win):
            self.col_base[w] = col
            wops = []
            lcb = 0
            for b in range(nb):
                nco = int(ncols[w, b])
                if nco:
                    wops.append((b, ioff, nco * P, lcb))
                    ioff += nco * P // 16
                    lcb += nco
                    col += nco
            self.ops.append(wops)
        self.col_base[self.n_win] = col
        self.total_cols = col
        self.idx_width = ioff
        self.wcols = np.diff(self.col_base).astype(np.int64)
        self.max_wcols = int(self.wcols.max()) if col else 0
        self.total_idx = col * P

        # per-core idx + masks
        self.idx16 = np.zeros((C, P, max(self.idx_width, 1)), np.int16)
        self.masks = np.zeros((C, P, max(col, 1), WD), F8)
        cell_base = {}
        lcb_map = {}
        for w in range(self.n_win):
            for (b, io, nidx, lcb) in self.ops[w]:
                cell_base[(w, b)] = io
                lcb_map[(w, b)] = self.col_base[w] + lcb
        for c in range(C):
            rows, dl, w_e, b_e = per_core[c]
            order = np.argsort(w_e * nb + b_e, kind="stable")
            rows_s, dl_s, w_s, b_s = rows[order], dl[order], w_e[order], b_e[order]
            rec_s = recip_dst_local[c][dl_s].astype(np.float32)
            rib_s = (rows_s % self.bank_rows).astype(np.int64)
            # rank within each (w, b) run
            key = w_s * nb + b_s
            # j = index within cell
            cellcnt = np.bincount(key, minlength=self.n_win * nb)
            starts = np.zeros(self.n_win * nb + 1, np.int64)
            np.cumsum(cellcnt, out=starts[1:])
            j = np.arange(len(key)) - starts[key]
            # idx slab (flat over ops)
            flat = np.zeros(max(self.idx_width, 1) * 16, np.int16)
            iobase = np.array([cell_base.get((w, b), -1) * 16
                               for w in range(self.n_win) for b in range(nb)]
                              ).reshape(self.n_win, nb)
            pos = iobase[w_s, b_s] + j
            flat[pos] = rib_s.astype(np.int16)
            w16 = flat.reshape(-1, 16).T       # [16, width]
            self.idx16[c] = np.tile(w16, (8, 1))
            # masks
            gcol = np.array([lcb_map.get((w, b), 0)
                             for w in range(self.n_win) for b in range(nb)]
                            ).reshape(self.n_win, nb)
            cc = gcol[w_s, b_s] + j // P
            pp = j % P
            off = dl_s - w_s * WD
            self.masks[c][pp, cc, off] = rec_s.astype(F8)


def _prep(inputs):
    f = lambda k: np.asarray(inputs[k], np.float32)
    x_author, x_paper = f("x_author"), f("x_paper")
    ws, wd = (np.asarray(inputs["ei_writes_src"], np.int64),
              np.asarray(inputs["ei_writes_dst"], np.int64))
    bs, bd = (np.asarray(inputs["ei_wb_src"], np.int64),
              np.asarray(inputs["ei_wb_dst"], np.int64))

    cnt_p = np.bincount(wd, minlength=NP_).astype(np.float32)
    cnt_a = np.bincount(bd, minlength=NA).astype(np.float32)
    recip_p = 1.0 / np.maximum(cnt_p, 1.0)
    recip_a = 1.0 / np.maximum(cnt_a, 1.0)

    # split edges by dst owner
    def split(src, dst, dst_can):
        srcs, dstls = [], []
        for c in range(C):
            m = (dst // dst_can) == c
            srcs.append(src[m])
            dstls.append((dst[m] % dst_can).astype(np.int64))
        return srcs, dstls

    w_src, w_dstl = split(ws, wd, P_CAN)     # writes: dst papers
    b_src, b_dstl = split(bs, bd, A_CAN)     # wb: dst authors

    # L1 compact tables (per-core unique srcs)
    uniqW = [np.unique(s) for s in w_src]    # authors referenced per core
    uniqB = [np.unique(s) for s in b_src]    # papers referenced per core
    rowsW = ((max(len(u) for u in uniqW) + P - 1) // P) * P
    rowsB = ((max(len(u) for u in uniqB) + P - 1) // P) * P
    assert rowsW <= 32768 and rowsB <= 32768
    xa_cmp = np.zeros((C, rowsW, IN), np.float16)
    xp_cmp = np.zeros((C, rowsB, IN), np.float16)
    for c in range(C):
        xa_cmp[c, :len(uniqW[c])] = x_author[uniqW[c]].astype(np.float16)
        xp_cmp[c, :len(uniqB[c])] = x_paper[uniqB[c]].astype(np.float16)

    recip_p_loc = [recip_p[c * P_CAN:(c + 1) * P_CAN] for c in range(C)]
    recip_a_loc = [recip_a[c * A_CAN:(c + 1) * A_CAN] for c in range(C)]

    # AG row mapping for L2 tables
    agW = [(s // A_CAN) * A_PAD + (s % A_CAN) for s in w_src]
    agB = [(s // P_CAN) * P_PAD + (s % P_CAN) for s in b_src]
    cmpW = [np.searchsorted(uniqW[c], w_src[c]) for c in range(C)]
    cmpB = [np.searchsorted(uniqB[c], b_src[c]) for c in range(C)]

    rels = dict(
        W1=RelLayer(cmpW, None, w_dstl, P_CAN, P_PAD, recip_p_loc, rowsW),
        B1=RelLayer(cmpB, None, b_dstl, A_CAN, A_PAD, recip_a_loc, rowsB),
        W2=RelLayer(agW, None, w_dstl, P_CAN, P_PAD, recip_p_loc, NA_AG),
        B2=RelLayer(agB, None, b_dstl, A_CAN, A_PAD, recip_a_loc, NP_AG),
    )

    # fp16 local chunks (root/skip transposes)
    xa_chunk = np.zeros((C, A_PAD, IN), np.float16)
    xp_chunk = np.zeros((C, P_PAD, IN), np.float16)
    for c in range(C):
        xa_chunk[c, :A_CAN] = x_author[c * A_CAN:(c + 1) * A_CAN]
        xp_chunk[c, :P_CAN] = x_paper[c * P_CAN:(c + 1) * P_CAN]

    # weight slab: 14 x [128, 256] fp16 (transposed: [in, out])
    wT = lambda k: f(k).T.astype(np.float16)
    slabs = [wT("c1w_Wl"), wT("c1w_Wr"), wT("c1b_Wl"), wT("c1b_Wr")]
    for k in ("c2w_Wl", "c2w_Wr", "c2b_Wl", "c2b_Wr"):
        w2 = wT(k)
        slabs += [w2[:128], w2[128:]]
    slabs += [wT("skipA_W"), wT("skipP_W")]
    wslab = np.concatenate(slabs, axis=0)          # [14*128, 256]

    pool_ones = np.zeros((P, 3), np.float16)
    pool_ones[:, 0] = 1.0
    pool_ones[:P_CAN - (P_PAD // P - 1) * P, 1] = 1.0   # last paper tile mask
    pool_ones[:A_CAN - (A_PAD // P - 1) * P, 2] = 1.0   # last author tile mask

    bias_nz = {k: bool(np.any(f(k))) for k in
               ("c1w_bl", "c1b_bl", "skipA_b", "skipP_b")}
    bias_arr = {k: np.broadcast_to(f(k2), (P, H)).astype(np.float32).copy()
                for k, k2 in (("bias_p1", "c1w_bl"), ("bias_a1", "c1b_bl"),
                              ("bias_p2", "skipP_b"), ("bias_a2", "skipA_b"))}

    in_maps = []
    for c in range(C):
        m = dict(
            xa_cmp=xa_cmp[c], xp_cmp=xp_cmp[c],
            xa_chunk=xa_chunk[c], xp_chunk=xp_chunk[c],
            wslab=wslab, pool_ones=pool_ones,
        )
        for nm, rl in rels.items():
            m["idx_" + nm] = rl.idx16[c]
            m["mask_" + nm] = rl.masks[c].reshape(P, -1)
        for k, arr in bias_arr.items():
            m[k] = arr
        in_maps.append(m)
    return rels, in_maps, bias_nz


def _build(rels, bias_nz, debug=False):
    nc = bacc.Bacc("TRN2", target_bir_lowering=False, debug=False,
                   num_devices=C)
    f16, f32, i16, f8 = dt.float16, dt.float32, dt.int16, dt.float8e4
    ein = lambda n, s, d: nc.dram_tensor(n, s, d, kind="ExternalInput")

    xa_cmp = ein("xa_cmp", [rels["W1"].table_rows, IN], f16)
    xp_cmp = ein("xp_cmp", [rels["B1"].table_rows, IN], f16)
    xa_chunk = ein("xa_chunk", [A_PAD, IN], f16)
    xp_chunk = ein("xp_chunk", [P_PAD, IN], f16)
    wslab = ein("wslab", [14 * P, H], f16)
    pool_in = ein("pool_ones", [P, 3], f16)
    idx_h, mask_h = {}, {}
    for nm, rl in rels.items():
        idx_h[nm] = ein("idx_" + nm, [P, max(rl.idx_width, 1)], i16)
        mask_h[nm] = ein("mask_" + nm, [P, max(rl.total_cols, 1) * WD], f8)
    bias_in = {k: ein(k, [P, H], f32)
               for k in ("bias_p1", "bias_a1", "bias_p2", "bias_a2")}

    out_pool = nc.dram_tensor("out_pool", [1, 2 * H], f32,
                              kind="ExternalOutput")
    if debug:
        dbg_h1a = nc.dram_tensor("dbg_h1a", [A_PAD, H], f16,
                                 kind="ExternalOutput")
        dbg_h1p = nc.dram_tensor("dbg_h1p", [P_PAD, H], f16,
                                 kind="ExternalOutput")

    W = {k: i for i, k in enumerate(
        ["c1w_Wl", "c1w_Wr", "c1b_Wl", "c1b_Wr",
         "c2w_Wl0", "c2w_Wl1", "c2w_Wr0", "c2w_Wr1",
         "c2b_Wl0", "c2b_Wl1", "c2b_Wr0", "c2b_Wr1",
         "skipA_W", "skipP_W"])}
    relu_f = mybir.ActivationFunctionType.Relu
    rg = [list(range(C))]
    MAXW = max(rl.max_wcols for rl in rels.values())

    with tile.TileContext(nc) as tc:
        with tc.tile_pool(name="persist", bufs=1) as pp, \
             tc.tile_pool(name="dram", bufs=1, space="DRAM") as dp, \
             tc.tile_pool(name="work", bufs=3) as wk, \
             tc.tile_pool(name="msgs", bufs=2) as mp, \
             tc.tile_pool(name="maskp", bufs=2) as mk, \
             tc.tile_pool(name="psA", bufs=4, space="PSUM") as psA, \
             tc.tile_pool(name="psL", bufs=2, space="PSUM") as psL, \
             tc.tile_pool(name="psP", bufs=1, space="PSUM") as psP:

            # ---------------- persistent loads
            wt = pp.tile([P, 14, H], f16, name="wt", tag="wt")
            nc.sync.dma_start(out=wt[:],
                              in_=wslab[:].rearrange("(s p) d -> p s d", p=P))
            pool_t = pp.tile([P, 3], f16, name="pool_t", tag="pool_t")
            nc.sync.dma_start(out=pool_t[:], in_=pool_in[:])
            idx_t = {}
            for nm, rl in rels.items():
                t = pp.tile([P, max(rl.idx_width, 1)], i16, name="idx" + nm)
                nc.sync.dma_start(out=t[:], in_=idx_h[nm][:])
                idx_t[nm] = t
            bias_t = {}
            for k, nz in (("bias_p1", bias_nz["c1w_bl"]),
                          ("bias_a1", bias_nz["c1b_bl"]),
                          ("bias_p2", bias_nz["skipP_b"]),
                          ("bias_a2", bias_nz["skipA_b"])):
                if nz:
                    t = pp.tile([P, H], f32, name=k + "_t")
                    nc.sync.dma_start(out=t[:], in_=bias_in[k][:])
                    bias_t[k] = t

            xaT = pp.tile([P, A_PAD], f16, name="xaT", tag="xaT")
            nc.sync.dma_start_transpose(out=xaT[:], in_=xa_chunk[:])
            xpT = pp.tile([P, P_PAD], f16, name="xpT", tag="xpT")
            nc.sync.dma_start_transpose(out=xpT[:], in_=xp_chunk[:])

            # h1 tables: fp8 local shard -> Shared-output AllGather table
            # (fp16 local shard feeds the DMA transposes for L2 root terms)
            h1a_sh = dp.tile([NA_AG, H], f8, name="h1a_sh", tag="h1a_sh",
                             addr_space="Shared")
            h1p_sh = dp.tile([NP_AG, H], f8, name="h1p_sh", tag="h1p_sh",
                             addr_space="Shared")
            h1a_l8 = dp.tile([A_PAD, H], f8, name="h1a_l8", tag="h1a_l8")
            h1p_l8 = dp.tile([P_PAD, H], f8, name="h1p_l8", tag="h1p_l8")
            h1a_loc = dp.tile([A_PAD, H], f16, name="h1a_loc", tag="h1a_loc")
            h1p_loc = dp.tile([P_PAD, H], f16, name="h1p_loc", tag="h1p_loc")

            def conv(nm, table, elem, Wl, Wr, rootT, skipW, skipT, bias,
                     h_l8, h_loc, pool_ps, pool_last_col):
                rl = rels[nm]
                nslice = elem // P
                it = idx_t[nm]
                mdt = f16 if nslice == 1 else f8
                for w in range(rl.n_win):
                    wc = int(rl.wcols[w])
                    cb = int(rl.col_base[w])
                    aggT = []
                    if wc:
                        msgs = mp.tile([P, MAXW, elem], mdt,
                                       tag=f"msgs{nslice}")
                        for (b, ioff, nidx, lcb) in rl.ops[w]:
                            b0 = b * rl.bank_rows
                            b1 = min(b0 + rl.bank_rows, rl.table_rows)
                            nc.gpsimd.dma_gather(
                                msgs[:, lcb:lcb + nidx // P, :elem],
                                table[b0:b1, :],
                                it[:, ioff:ioff + nidx // 16],
                                nidx, nidx, elem, single_packet=False)
                        mask_t = mk.tile([P, MAXW * WD], f8, tag="mask")
                        nc.scalar.dma_start(
                            out=mask_t[:, :wc * WD],
                            in_=mask_h[nm][:, cb * WD:(cb + wc) * WD])
                        aggs = []
                        for s in range(nslice):
                            aggs.append(psA.tile([P, WD], f32, tag="agg",
                                                 name="agg", space="PSUM"))
                        for i in range(wc):
                            for s in range(nslice):
                                nc.tensor.matmul(
                                    out=aggs[s][:],
                                    lhsT=msgs[:, i:i + 1, s * P:(s + 1) * P],
                                    rhs=mask_t[:, i * WD:(i + 1) * WD],
                                    start=(i == 0), stop=(i == wc - 1))
                        for s in range(nslice):
                            a = wk.tile([P, WD], f16, tag="aggT")
                            nc.scalar.copy(out=a[:], in_=aggs[s][:])
                            aggT.append(a)
                    for tl in range(min(WIN, rl.n_tiles - w * WIN)):
                        t = w * WIN + tl
                        lin = psL.tile([P, H], f32, tag="lin", space="PSUM")
                        first = True
                        if wc:
                            for s in range(nslice):
                                nc.tensor.matmul(
                                    out=lin[:],
                                    lhsT=aggT[s][:, tl * P:(tl + 1) * P],
                                    rhs=wt[:, Wl[s]:Wl[s] + 1, :],
                                    start=first, stop=False)
                                first = False
                        for s in range(nslice):
                            nc.tensor.matmul(
                                out=lin[:],
                                lhsT=rootT[s][:, t * P:(t + 1) * P],
                                rhs=wt[:, Wr[s]:Wr[s] + 1, :],
                                start=first,
                                stop=(skipW is None and s == nslice - 1))
                            first = False
                        if skipW is not None:
                            nc.tensor.matmul(
                                out=lin[:], lhsT=skipT[:, t * P:(t + 1) * P],
                                rhs=wt[:, skipW:skipW + 1, :],
                                start=False, stop=True)
                        h16 = wk.tile([P, H], f16, tag="h16")
                        if bias is None:
                            src = lin
                        else:
                            tmp = wk.tile([P, H], f32, tag="btmp")
                            nc.vector.tensor_add(out=tmp[:], in0=lin[:],
                                                 in1=bias[:])
                            src = tmp
                        nc.scalar.activation(out=h16[:], in_=src[:],
                                             func=relu_f)
                        if h_l8 is not None:
                            h8 = wk.tile([P, H], f8, tag="h8")
                            nc.scalar.activation(out=h8[:], in_=src[:],
                                                 func=relu_f)
                            nc.scalar.dma_start(
                                out=h_l8[t * P:(t + 1) * P, :], in_=h8[:])
                            nc.scalar.dma_start(
                                out=h_loc[t * P:(t + 1) * P, :], in_=h16[:])
                        if pool_ps is not None:
                            oc = pool_last_col if t == rl.n_tiles - 1 else 0
                            nc.tensor.matmul(
                                out=pool_ps[:], lhsT=pool_t[:, oc:oc + 1],
                                rhs=h16[:], start=(t == 0),
                                stop=(t == rl.n_tiles - 1),
                                skip_group_check=True)

            # -------- layer 1: authors (wb: src papers -> dst authors)
            conv("B1", xp_cmp, IN, [W["c1b_Wl"]], [W["c1b_Wr"]], [xaT],
                 None, None, bias_t.get("bias_a1"), h1a_l8, h1a_loc, None, 0)
            # issue collectives from the (idle) vector queue so the inline
            # completion wait never blocks the gpsimd gather stream
            nc.gpsimd.collective_compute(
                "AllGather", mybir.AluOpType.bypass, replica_groups=rg,
                ins=[h1a_l8.opt()], outs=[h1a_sh.opt()])

            # -------- layer 1: papers (writes: src authors -> dst papers)
            conv("W1", xa_cmp, IN, [W["c1w_Wl"]], [W["c1w_Wr"]], [xpT],
                 None, None, bias_t.get("bias_p1"), h1p_l8, h1p_loc, None, 0)

            # -------- transposed local h1 chunks (before the 2nd collective)
            h1aT = []
            for s in range(2):
                t = pp.tile([P, A_PAD], f16, name=f"h1aT{s}", tag=f"h1aT{s}")
                nc.sync.dma_start_transpose(
                    out=t[:], in_=h1a_loc[:, s * P:(s + 1) * P])
                h1aT.append(t)
            h1pT = []
            for s in range(2):
                t = pp.tile([P, P_PAD], f16, name=f"h1pT{s}", tag=f"h1pT{s}")
                nc.sync.dma_start_transpose(
                    out=t[:], in_=h1p_loc[:, s * P:(s + 1) * P])
                h1pT.append(t)

            nc.gpsimd.collective_compute(
                "AllGather", mybir.AluOpType.bypass, replica_groups=rg,
                ins=[h1p_l8.opt()], outs=[h1p_sh.opt()])

            # -------- layer 2: papers (gathers h1a from shared table)
            pool_p = psP.tile([1, H], f32, name="pool_p", tag="pool_p",
                              space="PSUM")
            pool_a = psP.tile([1, H], f32, name="pool_a", tag="pool_a",
                              space="PSUM")
            conv("W2", h1a_sh, H, [W["c2w_Wl0"], W["c2w_Wl1"]],
                 [W["c2w_Wr0"], W["c2w_Wr1"]], h1pT, W["skipP_W"], xpT,
                 bias_t.get("bias_p2"), None, None, pool_p, 1)

            # -------- layer 2: authors
            conv("B2", h1p_sh, H, [W["c2b_Wl0"], W["c2b_Wl1"]],
                 [W["c2b_Wr0"], W["c2b_Wr1"]], h1aT, W["skipA_W"], xaT,
                 bias_t.get("bias_a2"), None, None, pool_a, 2)

            pool_sb = wk.tile([1, 2 * H], f32, tag="poolout")
            nc.vector.tensor_copy(out=pool_sb[:, 0:H], in_=pool_a[:])
            nc.vector.tensor_copy(out=pool_sb[:, H:2 * H], in_=pool_p[:])
            nc.sync.dma_start(out=out_pool[:], in_=pool_sb[:])

            if debug:
                nc.sync.dma_start(out=dbg_h1a[:], in_=h1a_loc[:])
                nc.sync.dma_start(out=dbg_h1p[:], in_=h1p_loc[:])

    nc.compile()
    return nc


def kernel(**inputs):
    debug = bool(int(os.environ.get("GNN_DEBUG", "0")))
    trace = bool(int(os.environ.get("GNN_TRACE", "0")))
    rels, in_maps, bias_nz = _prep(inputs)
    nc = _build(rels, bias_nz, debug=debug)
    res = bass_utils.run_bass_kernel_spmd(
        nc, in_maps, core_ids=list(range(C)), trace=trace)
    kernel.last_results = res

    pools = np.stack([res.results[c]["out_pool"] for c in range(C)])
    sum_a = pools[:, 0, :H].astype(np.float64).sum(axis=0)
    sum_p = pools[:, 0, H:].astype(np.float64).sum(axis=0)
    pooled = np.concatenate([sum_a / NA, sum_p / NP_])[None, :]
    W1 = np.asarray(inputs["cls_W1"], np.float64)
    b1 = np.asarray(inputs["cls_b1"], np.float64)
    W2 = np.asarray(inputs["cls_W2"], np.float64)
    b2 = np.asarray(inputs["cls_b2"], np.float64)
    h = np.maximum(pooled @ W1.T + b1, 0.0)
    out = h @ W2.T + b2
    return out.astype(np.float32)


# revision 25
# speedup vs baseline: 1.4680x; 1.0377x over previous
"""Trainium2 Bass kernel for the GSAT HeteroGNN problem (8 NeuronCores).

Self-contained: hardcodes shapes/sharding; only imports the concourse
toolchain.

Strategy (dst-node sharding, SPMD over 8 cores):
  - papers split into 8 canonical chunks of 12500 (padded 12544 = 98 tiles),
    authors 8 x 6250 (padded 6272 = 49 tiles).
  - edges live on their dst's owner core, laid out host-side into 128-slot
    columns per (4-tile window, src-bank); dma_gather (int16 idx) fetches
    fp8 source rows as [128, cols, feat].
  - segment-mean via host-precomputed fp8 masks streamed by DMA:
    mask[slot, dst_in_window] = 1/deg(dst); TensorE accumulates
    aggT[feat, 512] in PSUM per window (no on-device mask building).
  - L1 gathers read per-core COMPACT fp8 tables (only the <=32k rows this
    core references -> single bank, minimal column padding).
  - L1 outputs h1 are written twice: fp8 rows into a local chunk that a
    Shared-output AllGather assembles into a shared fp8 table (each rank
    contributes only its 1.6-3.2MB shard; the old Local-output AllGathers
    moved 77MB/core), and fp16 into a local chunk used for DMA transposes
    (L2 root terms).
  - L2 gathers read the shared fp8 h1 tables directly.
  - all DMA transposes are placed before any collective in program order
    (the scheduler serializes transposes with collectives).
  - global mean-pool via ones-column matmuls accumulating in PSUM; final
    2-layer MLP on host in fp64.
"""
import os
import sys

try:
    import concourse  # noqa: F401
except ImportError:  # toolchain location in the grading container
    sys.path.insert(0, "/opt/trn_rl_repo")

import numpy as np
import ml_dtypes
from concourse import bass, bacc, mybir, tile  # noqa: F401
from concourse import bass_utils

dt = mybir.dt
F8 = ml_dtypes.float8_e4m3

# ---------------------------------------------------------------- constants
NA, NP_, E = 50000, 100000, 300000
IN, H, OUT = 128, 256, 16
C = 8                      # cores
P = 128                    # partitions
A_CAN, P_CAN = NA // C, NP_ // C              # 6250 / 12500
A_PAD = ((A_CAN + P - 1) // P) * P            # 6272
P_PAD = ((P_CAN + P - 1) // P) * P            # 12544
NA_AG, NP_AG = C * A_PAD, C * P_PAD           # 50176 / 100352
WIN = 4                    # dst tiles per PSUM window (512 dsts)
WD = WIN * P               # window width in dsts


class RelLayer:
    """Host-side layout for one (relation, layer): slot columns per
    (window, bank), uniform across cores (max-over-cores column counts),
    int16 gather indices and fp8 recip masks."""

    def __init__(self, row_of, dst_owner, dstl, n_dst_can, n_dst_pad,
                 recip_dst_local, table_rows):
        # row_of: [C] list of per-edge row ids (into this layer's table)
        # dst_owner/dstl: per-edge owner core and local dst id (global arrays
        # already split: row_of[c] aligned with dstl[c])
        self.n_tiles = n_dst_pad // P
        self.n_win = (self.n_tiles + WIN - 1) // WIN
        nb = (table_rows + 32767) // 32768
        self.n_banks = nb
        self.bank_rows = (table_rows + nb - 1) // nb
        self.table_rows = table_rows

        # per-core per-cell counts -> uniform ncols
        ncols = np.zeros((self.n_win, nb), np.int64)
        per_core = []
        for c in range(C):
            rows, dl = row_of[c], dstl[c]
            w = dl // WD
            b = rows // self.bank_rows
            cnt = np.zeros((self.n_win, nb), np.int64)
            np.add.at(cnt, (w, b), 1)
            ncols = np.maximum(ncols, (cnt + P - 1) // P)
            per_core.append((rows, dl, w, b))
        self.ncols = ncols

        # global column layout: window-major, bank-minor
        self.col_base = np.zeros(self.n_win + 1, np.int64)
        self.ops = []              # per window: list of (bank, ioff, nidx, lcb)
        ioff = 0
        col = 0
        for w in range(self.n_win):
            self.col_base[w] = col
            wops = []
            lcb = 0
            for b in range(nb):
                nco = int(ncols[w, b])
                if nco:
                    wops.append((b, ioff, nco * P, lcb))
                    ioff += nco * P // 16
                    lcb += nco
                    col += nco
            self.ops.append(wops)
        self.col_base[self.n_win] = col
        self.total_cols = col
        self.idx_width = ioff
        self.wcols = np.diff(self.col_base).astype(np.int64)
        self.max_wcols = int(self.wcols.max()) if col else 0
        self.total_idx = col * P

        # per-core idx + masks
        self.idx16 = np.zeros((C, P, max(self.idx_width, 1)), np.int16)
        self.masks = np.zeros((C, P, max(col, 1), WD), F8)
        cell_base = {}
        lcb_map = {}
        for w in range(self.n_win):
            for (b, io, nidx, lcb) in self.ops[w]:
                cell_base[(w, b)] = io
                lcb_map[(w, b)] = self.col_base[w] + lcb
        for c in range(C):
            rows, dl, w_e, b_e = per_core[c]
            order = np.argsort(w_e * nb + b_e, kind="stable")
            rows_s, dl_s, w_s, b_s = rows[order], dl[order], w_e[order], b_e[order]
            rec_s = recip_dst_local[c][dl_s].astype(np.float32)
            rib_s = (rows_s % self.bank_rows).astype(np.int64)
            # rank within each (w, b) run
            key = w_s * nb + b_s
            # j = index within cell
            cellcnt = np.bincount(key, minlength=self.n_win * nb)
            starts = np.zeros(self.n_win * nb + 1, np.int64)
            np.cumsum(cellcnt, out=starts[1:])
            j = np.arange(len(key)) - starts[key]
            # idx slab (flat over ops)
            flat = np.zeros(max(self.idx_width, 1) * 16, np.int16)
            iobase = np.array([cell_base.get((w, b), -1) * 16
                               for w in range(self.n_win) for b in range(nb)]
                              ).reshape(self.n_win, nb)
            pos = iobase[w_s, b_s] + j
            flat[pos] = rib_s.astype(np.int16)
            w16 = flat.reshape(-1, 16).T       # [16, width]
            self.idx16[c] = np.tile(w16, (8, 1))
            # masks
            gcol = np.array([lcb_map.get((w, b), 0)
                             for w in range(self.n_win) for b in range(nb)]
                            ).reshape(self.n_win, nb)
            cc = gcol[w_s, b_s] + j // P
            pp = j % P
            off = dl_s - w_s * WD
            self.masks[c][pp, cc, off] = rec_s.astype(F8)


def _balance_perm(deg, n_nodes, can):
    """Permutation node -> new global id, dealing nodes into (core, window)
    cells so per-cell degree sums are balanced (pool is perm-invariant)."""
    import heapq
    n_win = ((can + P - 1) // P + WIN - 1) // WIN
    caps, base = [], []
    for c in range(C):
        for w in range(n_win):
            cap = min(WD, can - w * WD)
            caps.append(cap)
            base.append(c * can + w * WD)
    order = np.argsort(-deg, kind="stable")
    heap = [(0.0, i) for i in range(len(caps))]
    heapq.heapify(heap)
    fill = np.zeros(len(caps), np.int64)
    perm = np.empty(n_nodes, np.int64)
    for nd in order:
        while True:
            s, i = heapq.heappop(heap)
            if fill[i] < caps[i]:
                break
        perm[nd] = base[i] + fill[i]
        fill[i] += 1
        if fill[i] < caps[i]:
            heapq.heappush(heap, (s + deg[nd], i))
    return perm


def _prep(inputs):
    f = lambda k: np.asarray(inputs[k], np.float32)
    x_author, x_paper = f("x_author"), f("x_paper")
    ws, wd = (np.asarray(inputs["ei_writes_src"], np.int64),
              np.asarray(inputs["ei_writes_dst"], np.int64))
    bs, bd = (np.asarray(inputs["ei_wb_src"], np.int64),
              np.asarray(inputs["ei_wb_dst"], np.int64))

    # relabel nodes so per-(core, window) dst-degree sums are balanced
    pa_perm = _balance_perm(np.bincount(wd, minlength=NP_), NP_, P_CAN)
    au_perm = _balance_perm(np.bincount(bd, minlength=NA), NA, A_CAN)
    inv_pa = np.empty(NP_, np.int64)
    inv_pa[pa_perm] = np.arange(NP_)
    inv_au = np.empty(NA, np.int64)
    inv_au[au_perm] = np.arange(NA)
    x_paper = x_paper[inv_pa]
    x_author = x_author[inv_au]
    wd, bs = pa_perm[wd], pa_perm[bs]
    ws, bd = au_perm[ws], au_perm[bd]

    cnt_p = np.bincount(wd, minlength=NP_).astype(np.float32)
    cnt_a = np.bincount(bd, minlength=NA).astype(np.float32)
    recip_p = 1.0 / np.maximum(cnt_p, 1.0)
    recip_a = 1.0 / np.maximum(cnt_a, 1.0)

    # split edges by dst owner
    def split(src, dst, dst_can):
        srcs, dstls = [], []
        for c in range(C):
            m = (dst // dst_can) == c
            srcs.append(src[m])
            dstls.append((dst[m] % dst_can).astype(np.int64))
        return srcs, dstls

    w_src, w_dstl = split(ws, wd, P_CAN)     # writes: dst papers
    b_src, b_dstl = split(bs, bd, A_CAN)     # wb: dst authors

    # L1 compact tables (per-core unique srcs)
    uniqW = [np.unique(s) for s in w_src]    # authors referenced per core
    uniqB = [np.unique(s) for s in b_src]    # papers referenced per core
    rowsW = ((max(len(u) for u in uniqW) + P - 1) // P) * P
    rowsB = ((max(len(u) for u in uniqB) + P - 1) // P) * P
    assert rowsW <= 32768 and rowsB <= 32768
    xa_cmp = np.zeros((C, rowsW, IN), np.float16)
    xp_cmp = np.zeros((C, rowsB, IN), np.float16)
    for c in range(C):
        xa_cmp[c, :len(uniqW[c])] = x_author[uniqW[c]].astype(np.float16)
        xp_cmp[c, :len(uniqB[c])] = x_paper[uniqB[c]].astype(np.float16)

    recip_p_loc = [recip_p[c * P_CAN:(c + 1) * P_CAN] for c in range(C)]
    recip_a_loc = [recip_a[c * A_CAN:(c + 1) * A_CAN] for c in range(C)]

    # AG row mapping for L2 tables
    agW = [(s // A_CAN) * A_PAD + (s % A_CAN) for s in w_src]
    agB = [(s // P_CAN) * P_PAD + (s % P_CAN) for s in b_src]
    cmpW = [np.searchsorted(uniqW[c], w_src[c]) for c in range(C)]
    cmpB = [np.searchsorted(uniqB[c], b_src[c]) for c in range(C)]

    rels = dict(
        W1=RelLayer(cmpW, None, w_dstl, P_CAN, P_PAD, recip_p_loc, rowsW),
        B1=RelLayer(cmpB, None, b_dstl, A_CAN, A_PAD, recip_a_loc, rowsB),
        W2=RelLayer(agW, None, w_dstl, P_CAN, P_PAD, recip_p_loc, NA_AG),
        B2=RelLayer(agB, None, b_dstl, A_CAN, A_PAD, recip_a_loc, NP_AG),
    )

    # fp16 local chunks (root/skip transposes)
    xa_chunk = np.zeros((C, A_PAD, IN), np.float16)
    xp_chunk = np.zeros((C, P_PAD, IN), np.float16)
    for c in range(C):
        xa_chunk[c, :A_CAN] = x_author[c * A_CAN:(c + 1) * A_CAN]
        xp_chunk[c, :P_CAN] = x_paper[c * P_CAN:(c + 1) * P_CAN]

    # weight slab: 14 x [128, 256] fp16 (transposed: [in, out])
    wT = lambda k: f(k).T.astype(np.float16)
    slabs = [wT("c1w_Wl"), wT("c1w_Wr"), wT("c1b_Wl"), wT("c1b_Wr")]
    for k in ("c2w_Wl", "c2w_Wr", "c2b_Wl", "c2b_Wr"):
        w2 = wT(k)
        slabs += [w2[:128], w2[128:]]
    slabs += [wT("skipA_W"), wT("skipP_W")]
    wslab = np.concatenate(slabs, axis=0)          # [14*128, 256]

    pool_ones = np.zeros((P, 3), np.float16)
    pool_ones[:, 0] = 1.0
    pool_ones[:P_CAN - (P_PAD // P - 1) * P, 1] = 1.0   # last paper tile mask
    pool_ones[:A_CAN - (A_PAD // P - 1) * P, 2] = 1.0   # last author tile mask

    bias_nz = {k: bool(np.any(f(k))) for k in
               ("c1w_bl", "c1b_bl", "skipA_b", "skipP_b")}
    bias_arr = {k: np.broadcast_to(f(k2), (P, H)).astype(np.float32).copy()
                for k, k2 in (("bias_p1", "c1w_bl"), ("bias_a1", "c1b_bl"),
                              ("bias_p2", "skipP_b"), ("bias_a2", "skipA_b"))}

    in_maps = []
    for c in range(C):
        m = dict(
            xa_cmp=xa_cmp[c], xp_cmp=xp_cmp[c],
            xa_chunk=xa_chunk[c], xp_chunk=xp_chunk[c],
            wslab=wslab, pool_ones=pool_ones,
        )
        for nm, rl in rels.items():
            m["idx_" + nm] = rl.idx16[c]
            m["mask_" + nm] = rl.masks[c].reshape(P, -1)
        for k, arr in bias_arr.items():
            m[k] = arr
        in_maps.append(m)
    return rels, in_maps, bias_nz


def _build(rels, bias_nz, debug=False):
    nc = bacc.Bacc("TRN2", target_bir_lowering=False, debug=False,
                   num_devices=C)
    f16, f32, i16, f8 = dt.float16, dt.float32, dt.int16, dt.float8e4
    ein = lambda n, s, d: nc.dram_tensor(n, s, d, kind="ExternalInput")

    xa_cmp = ein("xa_cmp", [rels["W1"].table_rows, IN], f16)
    xp_cmp = ein("xp_cmp", [rels["B1"].table_rows, IN], f16)
    xa_chunk = ein("xa_chunk", [A_PAD, IN], f16)
    xp_chunk = ein("xp_chunk", [P_PAD, IN], f16)
    wslab = ein("wslab", [14 * P, H], f16)
    pool_in = ein("pool_ones", [P, 3], f16)
    idx_h, mask_h = {}, {}
    for nm, rl in rels.items():
        idx_h[nm] = ein("idx_" + nm, [P, max(rl.idx_width, 1)], i16)
        mask_h[nm] = ein("mask_" + nm, [P, max(rl.total_cols, 1) * WD], f8)
    bias_in = {k: ein(k, [P, H], f32)
               for k in ("bias_p1", "bias_a1", "bias_p2", "bias_a2")}

    out_pool = nc.dram_tensor("out_pool", [1, 2 * H], f32,
                              kind="ExternalOutput")
    if debug:
        dbg_h1a = nc.dram_tensor("dbg_h1a", [A_PAD, H], f16,
                                 kind="ExternalOutput")
        dbg_h1p = nc.dram_tensor("dbg_h1p", [P_PAD, H], f16,
                                 kind="ExternalOutput")

    W = {k: i for i, k in enumerate(
        ["c1w_Wl", "c1w_Wr", "c1b_Wl", "c1b_Wr",
         "c2w_Wl0", "c2w_Wl1", "c2w_Wr0", "c2w_Wr1",
         "c2b_Wl0", "c2b_Wl1", "c2b_Wr0", "c2b_Wr1",
         "skipA_W", "skipP_W"])}
    relu_f = mybir.ActivationFunctionType.Relu
    rg = [list(range(C))]
    MAXW = max(rl.max_wcols for rl in rels.values())

    with tile.TileContext(nc) as tc:
        with tc.tile_pool(name="persist", bufs=1) as pp, \
             tc.tile_pool(name="dram", bufs=1, space="DRAM") as dp, \
             tc.tile_pool(name="work", bufs=3) as wk, \
             tc.tile_pool(name="msgs", bufs=2) as mp, \
             tc.tile_pool(name="maskp", bufs=2) as mk, \
             tc.tile_pool(name="psA", bufs=4, space="PSUM") as psA, \
             tc.tile_pool(name="psL", bufs=2, space="PSUM") as psL, \
             tc.tile_pool(name="psP", bufs=1, space="PSUM") as psP:

            # ---------------- persistent loads
            wt = pp.tile([P, 14, H], f16, name="wt", tag="wt")
            nc.sync.dma_start(out=wt[:],
                              in_=wslab[:].rearrange("(s p) d -> p s d", p=P))
            pool_t = pp.tile([P, 3], f16, name="pool_t", tag="pool_t")
            nc.sync.dma_start(out=pool_t[:], in_=pool_in[:])
            idx_t = {}
            for nm, rl in rels.items():
                t = pp.tile([P, max(rl.idx_width, 1)], i16, name="idx" + nm)
                nc.sync.dma_start(out=t[:], in_=idx_h[nm][:])
                idx_t[nm] = t
            bias_t = {}
            for k, nz in (("bias_p1", bias_nz["c1w_bl"]),
                          ("bias_a1", bias_nz["c1b_bl"]),
                          ("bias_p2", bias_nz["skipP_b"]),
                          ("bias_a2", bias_nz["skipA_b"])):
                if nz:
                    t = pp.tile([P, H], f32, name=k + "_t")
                    nc.sync.dma_start(out=t[:], in_=bias_in[k][:])
                    bias_t[k] = t

            xaT = pp.tile([P, A_PAD], f16, name="xaT", tag="xaT")
            nc.sync.dma_start_transpose(out=xaT[:], in_=xa_chunk[:])
            xpT = pp.tile([P, P_PAD], f16, name="xpT", tag="xpT")
            nc.sync.dma_start_transpose(out=xpT[:], in_=xp_chunk[:])

            # h1 tables: fp8 local shard -> Shared-output AllGather table
            # (fp16 local shard feeds the DMA transposes for L2 root terms)
            h1a_sh = dp.tile([NA_AG, H], f8, name="h1a_sh", tag="h1a_sh",
                             addr_space="Shared")
            h1p_sh = dp.tile([NP_AG, H], f8, name="h1p_sh", tag="h1p_sh",
                             addr_space="Shared")
            h1a_l8 = dp.tile([A_PAD, H], f8, name="h1a_l8", tag="h1a_l8")
            h1p_l8 = dp.tile([P_PAD, H], f8, name="h1p_l8", tag="h1p_l8")
            h1a_loc = dp.tile([A_PAD, H], f16, name="h1a_loc", tag="h1a_loc")
            h1p_loc = dp.tile([P_PAD, H], f16, name="h1p_loc", tag="h1p_loc")

            def conv(nm, table, elem, Wl, Wr, rootT, skipW, skipT, bias,
                     h_l8, h_loc, pool_ps, pool_last_col):
                rl = rels[nm]
                nslice = elem // P
                it = idx_t[nm]
                mdt = f16 if nslice == 1 else f8
                for w in range(rl.n_win):
                    wc = int(rl.wcols[w])
                    cb = int(rl.col_base[w])
                    aggT = []
                    if wc:
                        msgs = mp.tile([P, MAXW, elem], mdt,
                                       tag=f"msgs{nslice}")
                        for (b, ioff, nidx, lcb) in rl.ops[w]:
                            b0 = b * rl.bank_rows
                            b1 = min(b0 + rl.bank_rows, rl.table_rows)
                            nc.gpsimd.dma_gather(
                                msgs[:, lcb:lcb + nidx // P, :elem],
                                table[b0:b1, :],
                                it[:, ioff:ioff + nidx // 16],
                                nidx, nidx, elem, single_packet=False)
                        mask_t = mk.tile([P, MAXW * WD], f8, tag="mask")
                        nc.scalar.dma_start(
                            out=mask_t[:, :wc * WD],
                            in_=mask_h[nm][:, cb * WD:(cb + wc) * WD])
                        aggs = []
                        for s in range(nslice):
                            aggs.append(psA.tile([P, WD], f32, tag="agg",
                                                 name="agg", space="PSUM"))
                        for i in range(wc):
                            for s in range(nslice):
                                nc.tensor.matmul(
                                    out=aggs[s][:],
                                    lhsT=msgs[:, i:i + 1, s * P:(s + 1) * P],
                                    rhs=mask_t[:, i * WD:(i + 1) * WD],
                                    start=(i == 0), stop=(i == wc - 1))
                        for s in range(nslice):
                            a = wk.tile([P, WD], f16, tag="aggT")
                            nc.scalar.copy(out=a[:], in_=aggs[s][:])
                            aggT.append(a)
                    for tl in range(min(WIN, rl.n_tiles - w * WIN)):
                        t = w * WIN + tl
                        lin = psL.tile([P, H], f32, tag="lin", space="PSUM")
                        first = True
                        if wc:
                            for s in range(nslice):
                                nc.tensor.matmul(
                                    out=lin[:],
                                    lhsT=aggT[s][:, tl * P:(tl + 1) * P],
                                    rhs=wt[:, Wl[s]:Wl[s] + 1, :],
                                    start=first, stop=False)
                                first = False
                        for s in range(nslice):
                            nc.tensor.matmul(
                                out=lin[:],
                                lhsT=rootT[s][:, t * P:(t + 1) * P],
                                rhs=wt[:, Wr[s]:Wr[s] + 1, :],
                                start=first,
                                stop=(skipW is None and s == nslice - 1))
                            first = False
                        if skipW is not None:
                            nc.tensor.matmul(
                                out=lin[:], lhsT=skipT[:, t * P:(t + 1) * P],
                                rhs=wt[:, skipW:skipW + 1, :],
                                start=False, stop=True)
                        h16 = wk.tile([P, H], f16, tag="h16")
                        if bias is None:
                            src = lin
                        else:
                            tmp = wk.tile([P, H], f32, tag="btmp")
                            nc.vector.tensor_add(out=tmp[:], in0=lin[:],
                                                 in1=bias[:])
                            src = tmp
                        nc.scalar.activation(out=h16[:], in_=src[:],
                                             func=relu_f)
                        if h_l8 is not None:
                            h8 = wk.tile([P, H], f8, tag="h8")
                            nc.scalar.activation(out=h8[:], in_=src[:],
                                                 func=relu_f)
                            nc.scalar.dma_start(
                                out=h_l8[t * P:(t + 1) * P, :], in_=h8[:])
                            nc.scalar.dma_start(
                                out=h_loc[t * P:(t + 1) * P, :], in_=h16[:])
                        if pool_ps is not None:
                            oc = pool_last_col if t == rl.n_tiles - 1 else 0
                            nc.tensor.matmul(
                                out=pool_ps[:], lhsT=pool_t[:, oc:oc + 1],
                                rhs=h16[:], start=(t == 0),
                                stop=(t == rl.n_tiles - 1),
                                skip_group_check=True)

            # -------- layer 1: authors (wb: src papers -> dst authors)
            conv("B1", xp_cmp, IN, [W["c1b_Wl"]], [W["c1b_Wr"]], [xaT],
                 None, None, bias_t.get("bias_a1"), h1a_l8, h1a_loc, None, 0)
            # issue collectives from the (idle) vector queue so the inline
            # completion wait never blocks the gpsimd gather stream
            nc.gpsimd.collective_compute(
                "AllGather", mybir.AluOpType.bypass, replica_groups=rg,
                ins=[h1a_l8.opt()], outs=[h1a_sh.opt()])

            # -------- layer 1: papers (writes: src authors -> dst papers)
            conv("W1", xa_cmp, IN, [W["c1w_Wl"]], [W["c1w_Wr"]], [xpT],
                 None, None, bias_t.get("bias_p1"), h1p_l8, h1p_loc, None, 0)

            # -------- transposed local h1 chunks (before the 2nd collective)
            h1aT = []
            for s in range(2):
                t = pp.tile([P, A_PAD], f16, name=f"h1aT{s}", tag=f"h1aT{s}")
                nc.sync.dma_start_transpose(
                    out=t[:], in_=h1a_loc[:, s * P:(s + 1) * P])
                h1aT.append(t)
            h1pT = []
            for s in range(2):
                t = pp.tile([P, P_PAD], f16, name=f"h1pT{s}", tag=f"h1pT{s}")
                nc.sync.dma_start_transpose(
                    out=t[:], in_=h1p_loc[:, s * P:(s + 1) * P])
                h1pT.append(t)

            # -------- layer 2: papers (gathers h1a from shared table)
            pool_p = psP.tile([1, H], f32, name="pool_p", tag="pool_p",
                              space="PSUM")
            pool_a = psP.tile([1, H], f32, name="pool_a", tag="pool_a",
                              space="PSUM")
            conv("W2", h1a_sh, H, [W["c2w_Wl0"], W["c2w_Wl1"]],
                 [W["c2w_Wr0"], W["c2w_Wr1"]], h1pT, W["skipP_W"], xpT,
                 bias_t.get("bias_p2"), None, None, pool_p, 1)

            # AG(h1p) here: L2-papers does not consume it, so its inline
            # gpsimd wait lands after all L2-papers gathers are issued
            nc.gpsimd.collective_compute(
                "AllGather", mybir.AluOpType.bypass, replica_groups=rg,
                ins=[h1p_l8.opt()], outs=[h1p_sh.opt()])

            # -------- layer 2: authors
            conv("B2", h1p_sh, H, [W["c2b_Wl0"], W["c2b_Wl1"]],
                 [W["c2b_Wr0"], W["c2b_Wr1"]], h1aT, W["skipA_W"], xaT,
                 bias_t.get("bias_a2"), None, None, pool_a, 2)

            pool_sb = wk.tile([1, 2 * H], f32, tag="poolout")
            nc.vector.tensor_copy(out=pool_sb[:, 0:H], in_=pool_a[:])
            nc.vector.tensor_copy(out=pool_sb[:, H:2 * H], in_=pool_p[:])
            nc.sync.dma_start(out=out_pool[:], in_=pool_sb[:])

            if debug:
                nc.sync.dma_start(out=dbg_h1a[:], in_=h1a_loc[:])
                nc.sync.dma_start(out=dbg_h1p[:], in_=h1p_loc[:])

    nc.compile()
    return nc


def kernel(**inputs):
    debug = bool(int(os.environ.get("GNN_DEBUG", "0")))
    trace = bool(int(os.environ.get("GNN_TRACE", "0")))
    rels, in_maps, bias_nz = _prep(inputs)
    nc = _build(rels, bias_nz, debug=debug)
    res = bass_utils.run_bass_kernel_spmd(
        nc, in_maps, core_ids=list(range(C)), trace=trace)
    kernel.last_results = res

    pools = np.stack([res.results[c]["out_pool"] for c in range(C)])
    sum_a = pools[:, 0, :H].astype(np.float64).sum(axis=0)
    sum_p = pools[:, 0, H:].astype(np.float64).sum(axis=0)
    pooled = np.concatenate([sum_a / NA, sum_p / NP_])[None, :]
    W1 = np.asarray(inputs["cls_W1"], np.float64)
    b1 = np.asarray(inputs["cls_b1"], np.float64)
    W2 = np.asarray(inputs["cls_W2"], np.float64)
    b2 = np.asarray(inputs["cls_b2"], np.float64)
    h = np.maximum(pooled @ W1.T + b1, 0.0)
    out = h @ W2.T + b2
    return out.astype(np.float32)


# revision 26
# speedup vs baseline: 1.4870x; 1.0129x over previous
"""Trainium2 Bass kernel for the GSAT HeteroGNN problem (8 NeuronCores).

Self-contained: hardcodes shapes/sharding; only imports the concourse
toolchain.

Strategy (dst-node sharding, SPMD over 8 cores):
  - papers split into 8 canonical chunks of 12500 (padded 12544 = 98 tiles),
    authors 8 x 6250 (padded 6272 = 49 tiles).
  - edges live on their dst's owner core, laid out host-side into 128-slot
    columns per (4-tile window, src-bank); dma_gather (int16 idx) fetches
    fp8 source rows as [128, cols, feat].
  - segment-mean via host-precomputed fp8 masks streamed by DMA:
    mask[slot, dst_in_window] = 1/deg(dst); TensorE accumulates
    aggT[feat, 512] in PSUM per window (no on-device mask building).
  - L1 gathers read per-core COMPACT fp8 tables (only the <=32k rows this
    core references -> single bank, minimal column padding).
  - L1 outputs h1 are written twice: fp8 rows into a local chunk that a
    Shared-output AllGather assembles into a shared fp8 table (each rank
    contributes only its 1.6-3.2MB shard; the old Local-output AllGathers
    moved 77MB/core), and fp16 into a local chunk used for DMA transposes
    (L2 root terms).
  - L2 gathers read the shared fp8 h1 tables directly.
  - all DMA transposes are placed before any collective in program order
    (the scheduler serializes transposes with collectives).
  - global mean-pool via ones-column matmuls accumulating in PSUM; final
    2-layer MLP on host in fp64.
"""
import os
import sys

try:
    import concourse  # noqa: F401
except ImportError:  # toolchain location in the grading container
    sys.path.insert(0, "/opt/trn_rl_repo")

import numpy as np
import ml_dtypes
from concourse import bass, bacc, mybir, tile  # noqa: F401
from concourse import bass_utils
from concourse.bass import _add_dep_helper

dt = mybir.dt
F8 = ml_dtypes.float8_e4m3

# ---------------------------------------------------------------- constants
NA, NP_, E = 50000, 100000, 300000
IN, H, OUT = 128, 256, 16
C = 8                      # cores
P = 128                    # partitions
A_CAN, P_CAN = NA // C, NP_ // C              # 6250 / 12500
A_PAD = ((A_CAN + P - 1) // P) * P            # 6272
P_PAD = ((P_CAN + P - 1) // P) * P            # 12544
NA_AG, NP_AG = C * A_PAD, C * P_PAD           # 50176 / 100352
WIN = 4                    # dst tiles per PSUM window (512 dsts)
WD = WIN * P               # window width in dsts


class RelLayer:
    """Host-side layout for one (relation, layer): slot columns per
    (window, bank), uniform across cores (max-over-cores column counts),
    int16 gather indices and fp8 recip masks."""

    def __init__(self, row_of, dst_owner, dstl, n_dst_can, n_dst_pad,
                 recip_dst_local, table_rows):
        # row_of: [C] list of per-edge row ids (into this layer's table)
        # dst_owner/dstl: per-edge owner core and local dst id (global arrays
        # already split: row_of[c] aligned with dstl[c])
        self.n_tiles = n_dst_pad // P
        self.n_win = (self.n_tiles + WIN - 1) // WIN
        nb = (table_rows + 32767) // 32768
        self.n_banks = nb
        self.bank_rows = (table_rows + nb - 1) // nb
        self.table_rows = table_rows

        # per-core per-cell counts -> uniform ncols
        ncols = np.zeros((self.n_win, nb), np.int64)
        per_core = []
        for c in range(C):
            rows, dl = row_of[c], dstl[c]
            w = dl // WD
            b = rows // self.bank_rows
            cnt = np.zeros((self.n_win, nb), np.int64)
            np.add.at(cnt, (w, b), 1)
            ncols = np.maximum(ncols, (cnt + P - 1) // P)
            per_core.append((rows, dl, w, b))
        self.ncols = ncols

        # global column layout: window-major, bank-minor
        self.col_base = np.zeros(self.n_win + 1, np.int64)
        self.ops = []              # per window: list of (bank, ioff, nidx, lcb)
        ioff = 0
        col = 0
        for w in range(self.n_win):
            self.col_base[w] = col
            wops = []
            lcb = 0
            for b in range(nb):
                nco = int(ncols[w, b])
                if nco:
                    wops.append((b, ioff, nco * P, lcb))
                    ioff += nco * P // 16
                    lcb += nco
                    col += nco
            self.ops.append(wops)
        self.col_base[self.n_win] = col
        self.total_cols = col
        self.idx_width = ioff
        self.wcols = np.diff(self.col_base).astype(np.int64)
        self.max_wcols = int(self.wcols.max()) if col else 0
        self.total_idx = col * P

        # per-core idx + masks
        self.idx16 = np.zeros((C, P, max(self.idx_width, 1)), np.int16)
        self.masks = np.zeros((C, P, max(col, 1), WD), F8)
        cell_base = {}
        lcb_map = {}
        for w in range(self.n_win):
            for (b, io, nidx, lcb) in self.ops[w]:
                cell_base[(w, b)] = io
                lcb_map[(w, b)] = self.col_base[w] + lcb
        for c in range(C):
            rows, dl, w_e, b_e = per_core[c]
            order = np.argsort(w_e * nb + b_e, kind="stable")
            rows_s, dl_s, w_s, b_s = rows[order], dl[order], w_e[order], b_e[order]
            rec_s = recip_dst_local[c][dl_s].astype(np.float32)
            rib_s = (rows_s % self.bank_rows).astype(np.int64)
            # rank within each (w, b) run
            key = w_s * nb + b_s
            # j = index within cell
            cellcnt = np.bincount(key, minlength=self.n_win * nb)
            starts = np.zeros(self.n_win * nb + 1, np.int64)
            np.cumsum(cellcnt, out=starts[1:])
            j = np.arange(len(key)) - starts[key]
            # idx slab (flat over ops)
            flat = np.zeros(max(self.idx_width, 1) * 16, np.int16)
            iobase = np.array([cell_base.get((w, b), -1) * 16
                               for w in range(self.n_win) for b in range(nb)]
                              ).reshape(self.n_win, nb)
            pos = iobase[w_s, b_s] + j
            flat[pos] = rib_s.astype(np.int16)
            w16 = flat.reshape(-1, 16).T       # [16, width]
            self.idx16[c] = np.tile(w16, (8, 1))
            # masks
            gcol = np.array([lcb_map.get((w, b), 0)
                             for w in range(self.n_win) for b in range(nb)]
                            ).reshape(self.n_win, nb)
            cc = gcol[w_s, b_s] + j // P
            pp = j % P
            off = dl_s - w_s * WD
            self.masks[c][pp, cc, off] = rec_s.astype(F8)


def _balance_perm(deg, n_nodes, can):
    """Permutation node -> new global id, dealing nodes into (core, window)
    cells so per-cell degree sums are balanced (pool is perm-invariant)."""
    import heapq
    n_win = ((can + P - 1) // P + WIN - 1) // WIN
    caps, base = [], []
    for c in range(C):
        for w in range(n_win):
            cap = min(WD, can - w * WD)
            caps.append(cap)
            base.append(c * can + w * WD)
    order = np.argsort(-deg, kind="stable")
    heap = [(0.0, i) for i in range(len(caps))]
    heapq.heapify(heap)
    fill = np.zeros(len(caps), np.int64)
    perm = np.empty(n_nodes, np.int64)
    for nd in order:
        while True:
            s, i = heapq.heappop(heap)
            if fill[i] < caps[i]:
                break
        perm[nd] = base[i] + fill[i]
        fill[i] += 1
        if fill[i] < caps[i]:
            heapq.heappush(heap, (s + deg[nd], i))
    return perm


def _prep(inputs):
    f = lambda k: np.asarray(inputs[k], np.float32)
    x_author, x_paper = f("x_author"), f("x_paper")
    ws, wd = (np.asarray(inputs["ei_writes_src"], np.int64),
              np.asarray(inputs["ei_writes_dst"], np.int64))
    bs, bd = (np.asarray(inputs["ei_wb_src"], np.int64),
              np.asarray(inputs["ei_wb_dst"], np.int64))

    # relabel nodes so per-(core, window) dst-degree sums are balanced
    pa_perm = _balance_perm(np.bincount(wd, minlength=NP_), NP_, P_CAN)
    au_perm = _balance_perm(np.bincount(bd, minlength=NA), NA, A_CAN)
    inv_pa = np.empty(NP_, np.int64)
    inv_pa[pa_perm] = np.arange(NP_)
    inv_au = np.empty(NA, np.int64)
    inv_au[au_perm] = np.arange(NA)
    x_paper = x_paper[inv_pa]
    x_author = x_author[inv_au]
    wd, bs = pa_perm[wd], pa_perm[bs]
    ws, bd = au_perm[ws], au_perm[bd]

    cnt_p = np.bincount(wd, minlength=NP_).astype(np.float32)
    cnt_a = np.bincount(bd, minlength=NA).astype(np.float32)
    recip_p = 1.0 / np.maximum(cnt_p, 1.0)
    recip_a = 1.0 / np.maximum(cnt_a, 1.0)

    # split edges by dst owner
    def split(src, dst, dst_can):
        srcs, dstls = [], []
        for c in range(C):
            m = (dst // dst_can) == c
            srcs.append(src[m])
            dstls.append((dst[m] % dst_can).astype(np.int64))
        return srcs, dstls

    w_src, w_dstl = split(ws, wd, P_CAN)     # writes: dst papers
    b_src, b_dstl = split(bs, bd, A_CAN)     # wb: dst authors

    # L1 compact tables (per-core unique srcs)
    uniqW = [np.unique(s) for s in w_src]    # authors referenced per core
    uniqB = [np.unique(s) for s in b_src]    # papers referenced per core
    rowsW = ((max(len(u) for u in uniqW) + P - 1) // P) * P
    rowsB = ((max(len(u) for u in uniqB) + P - 1) // P) * P
    assert rowsW <= 32768 and rowsB <= 32768
    xa_cmp = np.zeros((C, rowsW, IN), np.float16)
    xp_cmp = np.zeros((C, rowsB, IN), np.float16)
    for c in range(C):
        xa_cmp[c, :len(uniqW[c])] = x_author[uniqW[c]].astype(np.float16)
        xp_cmp[c, :len(uniqB[c])] = x_paper[uniqB[c]].astype(np.float16)

    recip_p_loc = [recip_p[c * P_CAN:(c + 1) * P_CAN] for c in range(C)]
    recip_a_loc = [recip_a[c * A_CAN:(c + 1) * A_CAN] for c in range(C)]

    # AG row mapping for L2 tables
    agW = [(s // A_CAN) * A_PAD + (s % A_CAN) for s in w_src]
    agB = [(s // P_CAN) * P_PAD + (s % P_CAN) for s in b_src]
    cmpW = [np.searchsorted(uniqW[c], w_src[c]) for c in range(C)]
    cmpB = [np.searchsorted(uniqB[c], b_src[c]) for c in range(C)]

    rels = dict(
        W1=RelLayer(cmpW, None, w_dstl, P_CAN, P_PAD, recip_p_loc, rowsW),
        B1=RelLayer(cmpB, None, b_dstl, A_CAN, A_PAD, recip_a_loc, rowsB),
        W2=RelLayer(agW, None, w_dstl, P_CAN, P_PAD, recip_p_loc, NA_AG),
        B2=RelLayer(agB, None, b_dstl, A_CAN, A_PAD, recip_a_loc, NP_AG),
    )

    # fp16 local chunks (root/skip transposes)
    xa_chunk = np.zeros((C, A_PAD, IN), np.float16)
    xp_chunk = np.zeros((C, P_PAD, IN), np.float16)
    for c in range(C):
        xa_chunk[c, :A_CAN] = x_author[c * A_CAN:(c + 1) * A_CAN]
        xp_chunk[c, :P_CAN] = x_paper[c * P_CAN:(c + 1) * P_CAN]

    # weight slab: 14 x [128, 256] fp16 (transposed: [in, out])
    wT = lambda k: f(k).T.astype(np.float16)
    slabs = [wT("c1w_Wl"), wT("c1w_Wr"), wT("c1b_Wl"), wT("c1b_Wr")]
    for k in ("c2w_Wl", "c2w_Wr", "c2b_Wl", "c2b_Wr"):
        w2 = wT(k)
        slabs += [w2[:128], w2[128:]]
    slabs += [wT("skipA_W"), wT("skipP_W")]
    wslab = np.concatenate(slabs, axis=0)          # [14*128, 256]

    pool_ones = np.zeros((P, 3), np.float16)
    pool_ones[:, 0] = 1.0
    pool_ones[:P_CAN - (P_PAD // P - 1) * P, 1] = 1.0   # last paper tile mask
    pool_ones[:A_CAN - (A_PAD // P - 1) * P, 2] = 1.0   # last author tile mask

    bias_nz = {k: bool(np.any(f(k))) for k in
               ("c1w_bl", "c1b_bl", "skipA_b", "skipP_b")}
    bias_arr = {k: np.broadcast_to(f(k2), (P, H)).astype(np.float32).copy()
                for k, k2 in (("bias_p1", "c1w_bl"), ("bias_a1", "c1b_bl"),
                              ("bias_p2", "skipP_b"), ("bias_a2", "skipA_b"))}

    in_maps = []
    for c in range(C):
        m = dict(
            xa_cmp=xa_cmp[c], xp_cmp=xp_cmp[c],
            xa_chunk=xa_chunk[c], xp_chunk=xp_chunk[c],
            wslab=wslab, pool_ones=pool_ones,
        )
        for nm, rl in rels.items():
            m["idx_" + nm] = rl.idx16[c]
            m["mask_" + nm] = rl.masks[c].reshape(P, -1)
        for k, arr in bias_arr.items():
            m[k] = arr
        in_maps.append(m)
    return rels, in_maps, bias_nz


def _build(rels, bias_nz, debug=False):
    nc = bacc.Bacc("TRN2", target_bir_lowering=False, debug=False,
                   num_devices=C)
    f16, f32, i16, f8 = dt.float16, dt.float32, dt.int16, dt.float8e4
    ein = lambda n, s, d: nc.dram_tensor(n, s, d, kind="ExternalInput")

    xa_cmp = ein("xa_cmp", [rels["W1"].table_rows, IN], f16)
    xp_cmp = ein("xp_cmp", [rels["B1"].table_rows, IN], f16)
    xa_chunk = ein("xa_chunk", [A_PAD, IN], f16)
    xp_chunk = ein("xp_chunk", [P_PAD, IN], f16)
    wslab = ein("wslab", [14 * P, H], f16)
    pool_in = ein("pool_ones", [P, 3], f16)
    idx_h, mask_h = {}, {}
    for nm, rl in rels.items():
        idx_h[nm] = ein("idx_" + nm, [P, max(rl.idx_width, 1)], i16)
        mask_h[nm] = ein("mask_" + nm, [P, max(rl.total_cols, 1) * WD], f8)
    bias_in = {k: ein(k, [P, H], f32)
               for k in ("bias_p1", "bias_a1", "bias_p2", "bias_a2")}

    out_pool = nc.dram_tensor("out_pool", [1, 2 * H], f32,
                              kind="ExternalOutput")
    if debug:
        dbg_h1a = nc.dram_tensor("dbg_h1a", [A_PAD, H], f16,
                                 kind="ExternalOutput")
        dbg_h1p = nc.dram_tensor("dbg_h1p", [P_PAD, H], f16,
                                 kind="ExternalOutput")

    W = {k: i for i, k in enumerate(
        ["c1w_Wl", "c1w_Wr", "c1b_Wl", "c1b_Wr",
         "c2w_Wl0", "c2w_Wl1", "c2w_Wr0", "c2w_Wr1",
         "c2b_Wl0", "c2b_Wl1", "c2b_Wr0", "c2b_Wr1",
         "skipA_W", "skipP_W"])}
    relu_f = mybir.ActivationFunctionType.Relu
    rg = [list(range(C))]
    MAXW = max(rl.max_wcols for rl in rels.values())

    with tile.TileContext(nc) as tc:
        with tc.tile_pool(name="persist", bufs=1) as pp, \
             tc.tile_pool(name="dram", bufs=1, space="DRAM") as dp, \
             tc.tile_pool(name="work", bufs=3) as wk, \
             tc.tile_pool(name="msgs", bufs=2) as mp, \
             tc.tile_pool(name="maskp", bufs=2) as mk, \
             tc.tile_pool(name="psA", bufs=4, space="PSUM") as psA, \
             tc.tile_pool(name="psL", bufs=2, space="PSUM") as psL, \
             tc.tile_pool(name="psP", bufs=1, space="PSUM") as psP:

            # ---------------- persistent loads (idx first: gathers need it)
            idx_t = {}
            for nm in ("B1", "W1", "W2", "B2"):
                rl = rels[nm]
                t = pp.tile([P, max(rl.idx_width, 1)], i16, name="idx" + nm)
                nc.sync.dma_start(out=t[:], in_=idx_h[nm][:])
                idx_t[nm] = t
            wt = pp.tile([P, 14, H], f16, name="wt", tag="wt")
            nc.sync.dma_start(out=wt[:],
                              in_=wslab[:].rearrange("(s p) d -> p s d", p=P))
            pool_t = pp.tile([P, 3], f16, name="pool_t", tag="pool_t")
            nc.sync.dma_start(out=pool_t[:], in_=pool_in[:])
            bias_t = {}
            for k, nz in (("bias_p1", bias_nz["c1w_bl"]),
                          ("bias_a1", bias_nz["c1b_bl"]),
                          ("bias_p2", bias_nz["skipP_b"]),
                          ("bias_a2", bias_nz["skipA_b"])):
                if nz:
                    t = pp.tile([P, H], f32, name=k + "_t")
                    nc.sync.dma_start(out=t[:], in_=bias_in[k][:])
                    bias_t[k] = t

            xaT = pp.tile([P, A_PAD], f16, name="xaT", tag="xaT")
            nc.sync.dma_start_transpose(out=xaT[:], in_=xa_chunk[:])
            xpT = pp.tile([P, P_PAD], f16, name="xpT", tag="xpT")
            nc.sync.dma_start_transpose(out=xpT[:], in_=xp_chunk[:])

            # h1 tables: fp8 local shard -> Shared-output AllGather table
            # (fp16 local shard feeds the DMA transposes for L2 root terms)
            h1a_sh = dp.tile([NA_AG, H], f8, name="h1a_sh", tag="h1a_sh",
                             addr_space="Shared")
            h1p_sh = dp.tile([NP_AG, H], f8, name="h1p_sh", tag="h1p_sh",
                             addr_space="Shared")
            h1a_l8 = dp.tile([A_PAD, H], f8, name="h1a_l8", tag="h1a_l8")
            h1p_l8 = dp.tile([P_PAD, H], f8, name="h1p_l8", tag="h1p_l8")
            h1a_loc = dp.tile([A_PAD, H], f16, name="h1a_loc", tag="h1a_loc")
            h1p_loc = dp.tile([P_PAD, H], f16, name="h1p_loc", tag="h1p_loc")

            def conv(nm, table, elem, Wl, Wr, rootT, skipW, skipT, bias,
                     h_l8, h_loc, pool_ps, pool_last_col):
                rl = rels[nm]
                nslice = elem // P
                it = idx_t[nm]
                mdt = f16 if nslice == 1 else f8
                gathers = []
                for w in range(rl.n_win):
                    wc = int(rl.wcols[w])
                    cb = int(rl.col_base[w])
                    aggT = []
                    if wc:
                        msgs = mp.tile([P, MAXW, elem], mdt,
                                       tag=f"msgs{nslice}")
                        for (b, ioff, nidx, lcb) in rl.ops[w]:
                            b0 = b * rl.bank_rows
                            b1 = min(b0 + rl.bank_rows, rl.table_rows)
                            gathers.append(nc.gpsimd.dma_gather(
                                msgs[:, lcb:lcb + nidx // P, :elem],
                                table[b0:b1, :],
                                it[:, ioff:ioff + nidx // 16],
                                nidx, nidx, elem, single_packet=False))
                        mask_t = mk.tile([P, MAXW * WD], f8, tag="mask")
                        nc.scalar.dma_start(
                            out=mask_t[:, :wc * WD],
                            in_=mask_h[nm][:, cb * WD:(cb + wc) * WD])
                        aggs = []
                        for s in range(nslice):
                            aggs.append(psA.tile([P, WD], f32, tag="agg",
                                                 name="agg", space="PSUM"))
                        for i in range(wc):
                            for s in range(nslice):
                                nc.tensor.matmul(
                                    out=aggs[s][:],
                                    lhsT=msgs[:, i:i + 1, s * P:(s + 1) * P],
                                    rhs=mask_t[:, i * WD:(i + 1) * WD],
                                    start=(i == 0), stop=(i == wc - 1))
                        for s in range(nslice):
                            a = wk.tile([P, WD], f16, tag="aggT")
                            nc.scalar.copy(out=a[:], in_=aggs[s][:])
                            aggT.append(a)
                    for tl in range(min(WIN, rl.n_tiles - w * WIN)):
                        t = w * WIN + tl
                        lin = psL.tile([P, H], f32, tag="lin", space="PSUM")
                        first = True
                        if wc:
                            for s in range(nslice):
                                nc.tensor.matmul(
                                    out=lin[:],
                                    lhsT=aggT[s][:, tl * P:(tl + 1) * P],
                                    rhs=wt[:, Wl[s]:Wl[s] + 1, :],
                                    start=first, stop=False)
                                first = False
                        for s in range(nslice):
                            nc.tensor.matmul(
                                out=lin[:],
                                lhsT=rootT[s][:, t * P:(t + 1) * P],
                                rhs=wt[:, Wr[s]:Wr[s] + 1, :],
                                start=first,
                                stop=(skipW is None and s == nslice - 1))
                            first = False
                        if skipW is not None:
                            nc.tensor.matmul(
                                out=lin[:], lhsT=skipT[:, t * P:(t + 1) * P],
                                rhs=wt[:, skipW:skipW + 1, :],
                                start=False, stop=True)
                        h16 = wk.tile([P, H], f16, tag="h16")
                        if bias is None:
                            src = lin
                        else:
                            tmp = wk.tile([P, H], f32, tag="btmp")
                            nc.vector.tensor_add(out=tmp[:], in0=lin[:],
                                                 in1=bias[:])
                            src = tmp
                        nc.scalar.activation(out=h16[:], in_=src[:],
                                             func=relu_f)
                        if h_l8 is not None:
                            h8 = wk.tile([P, H], f8, tag="h8")
                            nc.scalar.activation(out=h8[:], in_=src[:],
                                                 func=relu_f)
                            nc.scalar.dma_start(
                                out=h_l8[t * P:(t + 1) * P, :], in_=h8[:])
                            nc.scalar.dma_start(
                                out=h_loc[t * P:(t + 1) * P, :], in_=h16[:])
                        if pool_ps is not None:
                            oc = pool_last_col if t == rl.n_tiles - 1 else 0
                            nc.tensor.matmul(
                                out=pool_ps[:], lhsT=pool_t[:, oc:oc + 1],
                                rhs=h16[:], start=(t == 0),
                                stop=(t == rl.n_tiles - 1),
                                skip_group_check=True)
                return gathers

            # -------- layer 1: authors (wb: src papers -> dst authors)
            conv("B1", xp_cmp, IN, [W["c1b_Wl"]], [W["c1b_Wr"]], [xaT],
                 None, None, bias_t.get("bias_a1"), h1a_l8, h1a_loc, None, 0)
            # issue collectives from the (idle) vector queue so the inline
            # completion wait never blocks the gpsimd gather stream
            nc.gpsimd.collective_compute(
                "AllGather", mybir.AluOpType.bypass, replica_groups=rg,
                ins=[h1a_l8.opt()], outs=[h1a_sh.opt()])

            # -------- layer 1: papers (writes: src authors -> dst papers)
            conv("W1", xa_cmp, IN, [W["c1w_Wl"]], [W["c1w_Wr"]], [xpT],
                 None, None, bias_t.get("bias_p1"), h1p_l8, h1p_loc, None, 0)

            # -------- transposed local h1 chunks (before the 2nd collective)
            h1aT = []
            for s in range(2):
                t = pp.tile([P, A_PAD], f16, name=f"h1aT{s}", tag=f"h1aT{s}")
                nc.sync.dma_start_transpose(
                    out=t[:], in_=h1a_loc[:, s * P:(s + 1) * P])
                h1aT.append(t)
            h1pT = []
            for s in range(2):
                t = pp.tile([P, P_PAD], f16, name=f"h1pT{s}", tag=f"h1pT{s}")
                nc.sync.dma_start_transpose(
                    out=t[:], in_=h1p_loc[:, s * P:(s + 1) * P])
                h1pT.append(t)

            # -------- layer 2: papers (gathers h1a from shared table)
            pool_p = psP.tile([1, H], f32, name="pool_p", tag="pool_p",
                              space="PSUM")
            pool_a = psP.tile([1, H], f32, name="pool_a", tag="pool_a",
                              space="PSUM")
            gW2 = conv("W2", h1a_sh, H, [W["c2w_Wl0"], W["c2w_Wl1"]],
                       [W["c2w_Wr0"], W["c2w_Wr1"]], h1pT, W["skipP_W"], xpT,
                       bias_t.get("bias_p2"), None, None, pool_p, 1)

            # AG(h1p): L2-papers does not consume it; pin it behind the last
            # L2-papers gather so the scheduler cannot hoist its inline wait
            # into the middle of the gather stream
            ccP = nc.gpsimd.collective_compute(
                "AllGather", mybir.AluOpType.bypass, replica_groups=rg,
                ins=[h1p_l8.opt()], outs=[h1p_sh.opt()])
            _add_dep_helper(ccP.ins, gW2[-1].ins,
                            reason="keep AG(h1p) after L2-papers gathers")

            # -------- layer 2: authors
            conv("B2", h1p_sh, H, [W["c2b_Wl0"], W["c2b_Wl1"]],
                 [W["c2b_Wr0"], W["c2b_Wr1"]], h1aT, W["skipA_W"], xaT,
                 bias_t.get("bias_a2"), None, None, pool_a, 2)

            pool_sb = wk.tile([1, 2 * H], f32, tag="poolout")
            nc.vector.tensor_copy(out=pool_sb[:, 0:H], in_=pool_a[:])
            nc.vector.tensor_copy(out=pool_sb[:, H:2 * H], in_=pool_p[:])
            nc.sync.dma_start(out=out_pool[:], in_=pool_sb[:])

            if debug:
                nc.sync.dma_start(out=dbg_h1a[:], in_=h1a_loc[:])
                nc.sync.dma_start(out=dbg_h1p[:], in_=h1p_loc[:])

    nc.compile()
    return nc


def kernel(**inputs):
    debug = bool(int(os.environ.get("GNN_DEBUG", "0")))
    trace = bool(int(os.environ.get("GNN_TRACE", "0")))
    rels, in_maps, bias_nz = _prep(inputs)
    nc = _build(rels, bias_nz, debug=debug)
    res = bass_utils.run_bass_kernel_spmd(
        nc, in_maps, core_ids=list(range(C)), trace=trace)
    kernel.last_results = res

    pools = np.stack([res.results[c]["out_pool"] for c in range(C)])
    sum_a = pools[:, 0, :H].astype(np.float64).sum(axis=0)
    sum_p = pools[:, 0, H:].astype(np.float64).sum(axis=0)
    pooled = np.concatenate([sum_a / NA, sum_p / NP_])[None, :]
    W1 = np.asarray(inputs["cls_W1"], np.float64)
    b1 = np.asarray(inputs["cls_b1"], np.float64)
    W2 = np.asarray(inputs["cls_W2"], np.float64)
    b2 = np.asarray(inputs["cls_b2"], np.float64)
    h = np.maximum(pooled @ W1.T + b1, 0.0)
    out = h @ W2.T + b2
    return out.astype(np.float32)


# revision 28
# speedup vs baseline: 1.4879x; 1.0006x over previous
"""Trainium2 Bass kernel for the GSAT HeteroGNN problem (8 NeuronCores).

Self-contained: hardcodes shapes/sharding; only imports the concourse
toolchain.

Strategy (dst-node sharding, SPMD over 8 cores):
  - papers split into 8 canonical chunks of 12500 (padded 12544 = 98 tiles),
    authors 8 x 6250 (padded 6272 = 49 tiles).
  - edges live on their dst's owner core, laid out host-side into 128-slot
    columns per (4-tile window, src-bank); dma_gather (int16 idx) fetches
    fp8 source rows as [128, cols, feat].
  - segment-mean via host-precomputed fp8 masks streamed by DMA:
    mask[slot, dst_in_window] = 1/deg(dst); TensorE accumulates
    aggT[feat, 512] in PSUM per window (no on-device mask building).
  - L1 gathers read per-core COMPACT fp8 tables (only the <=32k rows this
    core references -> single bank, minimal column padding).
  - L1 outputs h1 are written twice: fp8 rows into a local chunk that a
    Shared-output AllGather assembles into a shared fp8 table (each rank
    contributes only its 1.6-3.2MB shard; the old Local-output AllGathers
    moved 77MB/core), and fp16 into a local chunk used for DMA transposes
    (L2 root terms).
  - L2 gathers read the shared fp8 h1 tables directly.
  - all DMA transposes are placed before any collective in program order
    (the scheduler serializes transposes with collectives).
  - global mean-pool via ones-column matmuls accumulating in PSUM; final
    2-layer MLP on host in fp64.
"""
import os
import sys

try:
    import concourse  # noqa: F401
except ImportError:  # toolchain location in the grading container
    sys.path.insert(0, "/opt/trn_rl_repo")

import numpy as np
import ml_dtypes
from concourse import bass, bacc, mybir, tile  # noqa: F401
from concourse import bass_utils
from concourse.bass import _add_dep_helper

dt = mybir.dt
F8 = ml_dtypes.float8_e4m3

# ---------------------------------------------------------------- constants
NA, NP_, E = 50000, 100000, 300000
IN, H, OUT = 128, 256, 16
C = 8                      # cores
P = 128                    # partitions
A_CAN, P_CAN = NA // C, NP_ // C              # 6250 / 12500
A_PAD = ((A_CAN + P - 1) // P) * P            # 6272
P_PAD = ((P_CAN + P - 1) // P) * P            # 12544
NA_AG, NP_AG = C * A_PAD, C * P_PAD           # 50176 / 100352
WIN = 4                    # dst tiles per PSUM window (512 dsts)
WD = WIN * P               # window width in dsts


class RelLayer:
    """Host-side layout for one (relation, layer): slot columns per
    (window, bank), uniform across cores (max-over-cores column counts),
    int16 gather indices and fp8 recip masks."""

    def __init__(self, row_of, dst_owner, dstl, n_dst_can, n_dst_pad,
                 recip_dst_local, table_rows):
        # row_of: [C] list of per-edge row ids (into this layer's table)
        # dst_owner/dstl: per-edge owner core and local dst id (global arrays
        # already split: row_of[c] aligned with dstl[c])
        self.n_tiles = n_dst_pad // P
        self.n_win = (self.n_tiles + WIN - 1) // WIN
        nb = (table_rows + 32767) // 32768
        self.n_banks = nb
        self.bank_rows = (table_rows + nb - 1) // nb
        self.table_rows = table_rows

        # per-core per-cell counts -> uniform ncols
        ncols = np.zeros((self.n_win, nb), np.int64)
        per_core = []
        for c in range(C):
            rows, dl = row_of[c], dstl[c]
            w = dl // WD
            b = rows // self.bank_rows
            cnt = np.zeros((self.n_win, nb), np.int64)
            np.add.at(cnt, (w, b), 1)
            ncols = np.maximum(ncols, (cnt + P - 1) // P)
            per_core.append((rows, dl, w, b))
        self.ncols = ncols

        # global column layout: window-major, bank-minor
        self.col_base = np.zeros(self.n_win + 1, np.int64)
        self.ops = []              # per window: list of (bank, ioff, nidx, lcb)
        ioff = 0
        col = 0
        for w in range(self.n_win):
            self.col_base[w] = col
            wops = []
            lcb = 0
            for b in range(nb):
                nco = int(ncols[w, b])
                if nco:
                    wops.append((b, ioff, nco * P, lcb))
                    ioff += nco * P // 16
                    lcb += nco
                    col += nco
            self.ops.append(wops)
        self.col_base[self.n_win] = col
        self.total_cols = col
        self.idx_width = ioff
        self.wcols = np.diff(self.col_base).astype(np.int64)
        self.max_wcols = int(self.wcols.max()) if col else 0
        self.total_idx = col * P

        # per-core idx + masks
        self.idx16 = np.zeros((C, P, max(self.idx_width, 1)), np.int16)
        self.masks = np.zeros((C, P, max(col, 1), WD), F8)
        cell_base = {}
        lcb_map = {}
        for w in range(self.n_win):
            for (b, io, nidx, lcb) in self.ops[w]:
                cell_base[(w, b)] = io
                lcb_map[(w, b)] = self.col_base[w] + lcb
        for c in range(C):
            rows, dl, w_e, b_e = per_core[c]
            order = np.argsort(w_e * nb + b_e, kind="stable")
            rows_s, dl_s, w_s, b_s = rows[order], dl[order], w_e[order], b_e[order]
            rec_s = recip_dst_local[c][dl_s].astype(np.float32)
            rib_s = (rows_s % self.bank_rows).astype(np.int64)
            # rank within each (w, b) run
            key = w_s * nb + b_s
            # j = index within cell
            cellcnt = np.bincount(key, minlength=self.n_win * nb)
            starts = np.zeros(self.n_win * nb + 1, np.int64)
            np.cumsum(cellcnt, out=starts[1:])
            j = np.arange(len(key)) - starts[key]
            # idx slab (flat over ops)
            flat = np.zeros(max(self.idx_width, 1) * 16, np.int16)
            iobase = np.array([cell_base.get((w, b), -1) * 16
                               for w in range(self.n_win) for b in range(nb)]
                              ).reshape(self.n_win, nb)
            pos = iobase[w_s, b_s] + j
            flat[pos] = rib_s.astype(np.int16)
            w16 = flat.reshape(-1, 16).T       # [16, width]
            self.idx16[c] = np.tile(w16, (8, 1))
            # masks
            gcol = np.array([lcb_map.get((w, b), 0)
                             for w in range(self.n_win) for b in range(nb)]
                            ).reshape(self.n_win, nb)
            cc = gcol[w_s, b_s] + j // P
            pp = j % P
            off = dl_s - w_s * WD
            self.masks[c][pp, cc, off] = rec_s.astype(F8)


def _balance_perm(deg, n_nodes, can):
    """Permutation node -> new global id, dealing nodes into (core, window)
    cells so per-cell degree sums are balanced (pool is perm-invariant)."""
    import heapq
    n_win = ((can + P - 1) // P + WIN - 1) // WIN
    caps, base = [], []
    for c in range(C):
        for w in range(n_win):
            cap = min(WD, can - w * WD)
            caps.append(cap)
            base.append(c * can + w * WD)
    order = np.argsort(-deg, kind="stable")
    heap = [(0.0, i) for i in range(len(caps))]
    heapq.heapify(heap)
    fill = np.zeros(len(caps), np.int64)
    perm = np.empty(n_nodes, np.int64)
    for nd in order:
        while True:
            s, i = heapq.heappop(heap)
            if fill[i] < caps[i]:
                break
        perm[nd] = base[i] + fill[i]
        fill[i] += 1
        if fill[i] < caps[i]:
            heapq.heappush(heap, (s + deg[nd], i))
    return perm


def _prep(inputs):
    f = lambda k: np.asarray(inputs[k], np.float32)
    x_author, x_paper = f("x_author"), f("x_paper")
    ws, wd = (np.asarray(inputs["ei_writes_src"], np.int64),
              np.asarray(inputs["ei_writes_dst"], np.int64))
    bs, bd = (np.asarray(inputs["ei_wb_src"], np.int64),
              np.asarray(inputs["ei_wb_dst"], np.int64))

    # relabel nodes so per-(core, window) dst-degree sums are balanced
    pa_perm = _balance_perm(np.bincount(wd, minlength=NP_), NP_, P_CAN)
    au_perm = _balance_perm(np.bincount(bd, minlength=NA), NA, A_CAN)
    inv_pa = np.empty(NP_, np.int64)
    inv_pa[pa_perm] = np.arange(NP_)
    inv_au = np.empty(NA, np.int64)
    inv_au[au_perm] = np.arange(NA)
    x_paper = x_paper[inv_pa]
    x_author = x_author[inv_au]
    wd, bs = pa_perm[wd], pa_perm[bs]
    ws, bd = au_perm[ws], au_perm[bd]

    cnt_p = np.bincount(wd, minlength=NP_).astype(np.float32)
    cnt_a = np.bincount(bd, minlength=NA).astype(np.float32)
    recip_p = 1.0 / np.maximum(cnt_p, 1.0)
    recip_a = 1.0 / np.maximum(cnt_a, 1.0)

    # split edges by dst owner
    def split(src, dst, dst_can):
        srcs, dstls = [], []
        for c in range(C):
            m = (dst // dst_can) == c
            srcs.append(src[m])
            dstls.append((dst[m] % dst_can).astype(np.int64))
        return srcs, dstls

    w_src, w_dstl = split(ws, wd, P_CAN)     # writes: dst papers
    b_src, b_dstl = split(bs, bd, A_CAN)     # wb: dst authors

    # L1 compact tables (per-core unique srcs)
    uniqW = [np.unique(s) for s in w_src]    # authors referenced per core
    uniqB = [np.unique(s) for s in b_src]    # papers referenced per core
    rowsW = ((max(len(u) for u in uniqW) + P - 1) // P) * P
    rowsB = ((max(len(u) for u in uniqB) + P - 1) // P) * P
    assert rowsW <= 32768 and rowsB <= 32768
    xa_cmp = np.zeros((C, rowsW, IN), np.float16)
    xp_cmp = np.zeros((C, rowsB, IN), np.float16)
    for c in range(C):
        xa_cmp[c, :len(uniqW[c])] = x_author[uniqW[c]].astype(np.float16)
        xp_cmp[c, :len(uniqB[c])] = x_paper[uniqB[c]].astype(np.float16)

    recip_p_loc = [recip_p[c * P_CAN:(c + 1) * P_CAN] for c in range(C)]
    recip_a_loc = [recip_a[c * A_CAN:(c + 1) * A_CAN] for c in range(C)]

    # AG row mapping for L2 tables
    agW = [(s // A_CAN) * A_PAD + (s % A_CAN) for s in w_src]
    agB = [(s // P_CAN) * P_PAD + (s % P_CAN) for s in b_src]
    cmpW = [np.searchsorted(uniqW[c], w_src[c]) for c in range(C)]
    cmpB = [np.searchsorted(uniqB[c], b_src[c]) for c in range(C)]

    rels = dict(
        W1=RelLayer(cmpW, None, w_dstl, P_CAN, P_PAD, recip_p_loc, rowsW),
        B1=RelLayer(cmpB, None, b_dstl, A_CAN, A_PAD, recip_a_loc, rowsB),
        W2=RelLayer(agW, None, w_dstl, P_CAN, P_PAD, recip_p_loc, NA_AG),
        B2=RelLayer(agB, None, b_dstl, A_CAN, A_PAD, recip_a_loc, NP_AG),
    )

    # fp16 local chunks (root/skip transposes)
    xa_chunk = np.zeros((C, A_PAD, IN), np.float16)
    xp_chunk = np.zeros((C, P_PAD, IN), np.float16)
    for c in range(C):
        xa_chunk[c, :A_CAN] = x_author[c * A_CAN:(c + 1) * A_CAN]
        xp_chunk[c, :P_CAN] = x_paper[c * P_CAN:(c + 1) * P_CAN]

    # weight slab: 14 x [128, 256] fp16 (transposed: [in, out])
    wT = lambda k: f(k).T.astype(np.float16)
    slabs = [wT("c1w_Wl"), wT("c1w_Wr"), wT("c1b_Wl"), wT("c1b_Wr")]
    for k in ("c2w_Wl", "c2w_Wr", "c2b_Wl", "c2b_Wr"):
        w2 = wT(k)
        slabs += [w2[:128], w2[128:]]
    slabs += [wT("skipA_W"), wT("skipP_W")]
    wslab = np.concatenate(slabs, axis=0)          # [14*128, 256]

    pool_ones = np.zeros((P, 3), np.float16)
    pool_ones[:, 0] = 1.0
    pool_ones[:P_CAN - (P_PAD // P - 1) * P, 1] = 1.0   # last paper tile mask
    pool_ones[:A_CAN - (A_PAD // P - 1) * P, 2] = 1.0   # last author tile mask

    bias_nz = {k: bool(np.any(f(k))) for k in
               ("c1w_bl", "c1b_bl", "skipA_b", "skipP_b")}
    bias_arr = {k: np.broadcast_to(f(k2), (P, H)).astype(np.float32).copy()
                for k, k2 in (("bias_p1", "c1w_bl"), ("bias_a1", "c1b_bl"),
                              ("bias_p2", "skipP_b"), ("bias_a2", "skipA_b"))}

    in_maps = []
    for c in range(C):
        m = dict(
            xa_cmp=xa_cmp[c], xp_cmp=xp_cmp[c],
            xa_chunk=xa_chunk[c], xp_chunk=xp_chunk[c],
            wslab=wslab, pool_ones=pool_ones,
        )
        for nm, rl in rels.items():
            m["idx_" + nm] = rl.idx16[c]
            m["mask_" + nm] = rl.masks[c].reshape(P, -1)
        for k, arr in bias_arr.items():
            m[k] = arr
        in_maps.append(m)
    return rels, in_maps, bias_nz


def _build(rels, bias_nz, debug=False):
    nc = bacc.Bacc("TRN2", target_bir_lowering=False, debug=False,
                   num_devices=C)
    f16, f32, i16, f8 = dt.float16, dt.float32, dt.int16, dt.float8e4
    ein = lambda n, s, d: nc.dram_tensor(n, s, d, kind="ExternalInput")

    xa_cmp = ein("xa_cmp", [rels["W1"].table_rows, IN], f16)
    xp_cmp = ein("xp_cmp", [rels["B1"].table_rows, IN], f16)
    xa_chunk = ein("xa_chunk", [A_PAD, IN], f16)
    xp_chunk = ein("xp_chunk", [P_PAD, IN], f16)
    wslab = ein("wslab", [14 * P, H], f16)
    pool_in = ein("pool_ones", [P, 3], f16)
    idx_h, mask_h = {}, {}
    for nm, rl in rels.items():
        idx_h[nm] = ein("idx_" + nm, [P, max(rl.idx_width, 1)], i16)
        mask_h[nm] = ein("mask_" + nm, [P, max(rl.total_cols, 1) * WD], f8)
    bias_in = {k: ein(k, [P, H], f32)
               for k in ("bias_p1", "bias_a1", "bias_p2", "bias_a2")}

    out_pool = nc.dram_tensor("out_pool", [1, 2 * H], f32,
                              kind="ExternalOutput")
    if debug:
        dbg_h1a = nc.dram_tensor("dbg_h1a", [A_PAD, H], f16,
                                 kind="ExternalOutput")
        dbg_h1p = nc.dram_tensor("dbg_h1p", [P_PAD, H], f16,
                                 kind="ExternalOutput")

    W = {k: i for i, k in enumerate(
        ["c1w_Wl", "c1w_Wr", "c1b_Wl", "c1b_Wr",
         "c2w_Wl0", "c2w_Wl1", "c2w_Wr0", "c2w_Wr1",
         "c2b_Wl0", "c2b_Wl1", "c2b_Wr0", "c2b_Wr1",
         "skipA_W", "skipP_W"])}
    relu_f = mybir.ActivationFunctionType.Relu
    rg = [list(range(C))]
    MAXW = max(rl.max_wcols for rl in rels.values())

    with tile.TileContext(nc) as tc:
        with tc.tile_pool(name="persist", bufs=1) as pp, \
             tc.tile_pool(name="dram", bufs=1, space="DRAM") as dp, \
             tc.tile_pool(name="work", bufs=3) as wk, \
             tc.tile_pool(name="msgs", bufs=2) as mp, \
             tc.tile_pool(name="maskp", bufs=2) as mk, \
             tc.tile_pool(name="psA", bufs=4, space="PSUM") as psA, \
             tc.tile_pool(name="psL", bufs=2, space="PSUM") as psL, \
             tc.tile_pool(name="psP", bufs=1, space="PSUM") as psP:

            # ---------------- persistent loads (idx first: gathers need it)
            idx_t = {}
            for nm in ("B1", "W1", "W2", "B2"):
                rl = rels[nm]
                t = pp.tile([P, max(rl.idx_width, 1)], i16, name="idx" + nm)
                nc.sync.dma_start(out=t[:], in_=idx_h[nm][:])
                idx_t[nm] = t
            wt = pp.tile([P, 14, H], f16, name="wt", tag="wt")
            nc.sync.dma_start(out=wt[:],
                              in_=wslab[:].rearrange("(s p) d -> p s d", p=P))
            pool_t = pp.tile([P, 3], f16, name="pool_t", tag="pool_t")
            nc.sync.dma_start(out=pool_t[:], in_=pool_in[:])
            bias_t = {}
            for k, nz in (("bias_p1", bias_nz["c1w_bl"]),
                          ("bias_a1", bias_nz["c1b_bl"]),
                          ("bias_p2", bias_nz["skipP_b"]),
                          ("bias_a2", bias_nz["skipA_b"])):
                if nz:
                    t = pp.tile([P, H], f32, name=k + "_t")
                    nc.sync.dma_start(out=t[:], in_=bias_in[k][:])
                    bias_t[k] = t

            xaT = pp.tile([P, A_PAD], f16, name="xaT", tag="xaT")
            nc.sync.dma_start_transpose(out=xaT[:], in_=xa_chunk[:])
            xpT = pp.tile([P, P_PAD], f16, name="xpT", tag="xpT")
            nc.sync.dma_start_transpose(out=xpT[:], in_=xp_chunk[:])

            # h1 tables: fp8 local shard -> Shared-output AllGather table
            # (fp16 local shard feeds the DMA transposes for L2 root terms)
            h1a_sh = dp.tile([NA_AG, H], f8, name="h1a_sh", tag="h1a_sh",
                             addr_space="Shared")
            h1p_sh = dp.tile([NP_AG, H], f8, name="h1p_sh", tag="h1p_sh",
                             addr_space="Shared")
            h1a_l8 = dp.tile([A_PAD, H], f8, name="h1a_l8", tag="h1a_l8")
            h1p_l8 = dp.tile([P_PAD, H], f8, name="h1p_l8", tag="h1p_l8")
            h1a_loc = dp.tile([A_PAD, H], f16, name="h1a_loc", tag="h1a_loc")
            h1p_loc = dp.tile([P_PAD, H], f16, name="h1p_loc", tag="h1p_loc")

            def conv(nm, table, elem, Wl, Wr, rootT, skipW, skipT, bias,
                     h_l8, h_loc, pool_ps, pool_last_col):
                rl = rels[nm]
                nslice = elem // P
                it = idx_t[nm]
                mdt = f16 if nslice == 1 else f8
                gathers = []
                for w in range(rl.n_win):
                    wc = int(rl.wcols[w])
                    cb = int(rl.col_base[w])
                    aggT = []
                    if wc:
                        msgs = mp.tile([P, MAXW, elem], mdt,
                                       tag=f"msgs{nslice}")
                        for (b, ioff, nidx, lcb) in rl.ops[w]:
                            b0 = b * rl.bank_rows
                            b1 = min(b0 + rl.bank_rows, rl.table_rows)
                            gathers.append(nc.gpsimd.dma_gather(
                                msgs[:, lcb:lcb + nidx // P, :elem],
                                table[b0:b1, :],
                                it[:, ioff:ioff + nidx // 16],
                                nidx, nidx, elem, single_packet=False))
                        mask_t = mk.tile([P, MAXW * WD], f8, tag="mask")
                        nc.scalar.dma_start(
                            out=mask_t[:, :wc * WD],
                            in_=mask_h[nm][:, cb * WD:(cb + wc) * WD])
                        aggs = []
                        for s in range(nslice):
                            aggs.append(psA.tile([P, WD], f32, tag="agg",
                                                 name="agg", space="PSUM"))
                        for i in range(wc):
                            for s in range(nslice):
                                nc.tensor.matmul(
                                    out=aggs[s][:],
                                    lhsT=msgs[:, i:i + 1, s * P:(s + 1) * P],
                                    rhs=mask_t[:, i * WD:(i + 1) * WD],
                                    start=(i == 0), stop=(i == wc - 1))
                        for s in range(nslice):
                            a = wk.tile([P, WD], f16, tag="aggT")
                            nc.scalar.copy(out=a[:], in_=aggs[s][:])
                            aggT.append(a)
                    for tl in range(min(WIN, rl.n_tiles - w * WIN)):
                        t = w * WIN + tl
                        lin = psL.tile([P, H], f32, tag="lin", space="PSUM")
                        first = True
                        if wc:
                            for s in range(nslice):
                                nc.tensor.matmul(
                                    out=lin[:],
                                    lhsT=aggT[s][:, tl * P:(tl + 1) * P],
                                    rhs=wt[:, Wl[s]:Wl[s] + 1, :],
                                    start=first, stop=False)
                                first = False
                        for s in range(nslice):
                            nc.tensor.matmul(
                                out=lin[:],
                                lhsT=rootT[s][:, t * P:(t + 1) * P],
                                rhs=wt[:, Wr[s]:Wr[s] + 1, :],
                                start=first,
                                stop=(skipW is None and s == nslice - 1))
                            first = False
                        if skipW is not None:
                            nc.tensor.matmul(
                                out=lin[:], lhsT=skipT[:, t * P:(t + 1) * P],
                                rhs=wt[:, skipW:skipW + 1, :],
                                start=False, stop=True)
                        h16 = wk.tile([P, H], f16, tag="h16")
                        if bias is None:
                            src = lin
                        else:
                            tmp = wk.tile([P, H], f32, tag="btmp")
                            nc.vector.tensor_add(out=tmp[:], in0=lin[:],
                                                 in1=bias[:])
                            src = tmp
                        nc.scalar.activation(out=h16[:], in_=src[:],
                                             func=relu_f)
                        if h_l8 is not None:
                            h8 = wk.tile([P, H], f8, tag="h8")
                            nc.scalar.activation(out=h8[:], in_=src[:],
                                                 func=relu_f)
                            nc.scalar.dma_start(
                                out=h_l8[t * P:(t + 1) * P, :], in_=h8[:])
                            nc.scalar.dma_start(
                                out=h_loc[t * P:(t + 1) * P, :], in_=h16[:])
                        if pool_ps is not None:
                            oc = pool_last_col if t == rl.n_tiles - 1 else 0
                            nc.tensor.matmul(
                                out=pool_ps[:], lhsT=pool_t[:, oc:oc + 1],
                                rhs=h16[:], start=(t == 0),
                                stop=(t == rl.n_tiles - 1),
                                skip_group_check=True)
                return gathers

            # -------- layer 1: authors (wb: src papers -> dst authors)
            conv("B1", xp_cmp, IN, [W["c1b_Wl"]], [W["c1b_Wr"]], [xaT],
                 None, None, bias_t.get("bias_a1"), h1a_l8, h1a_loc, None, 0)
            h1aT = []
            for s in range(2):
                t = pp.tile([P, A_PAD], f16, name=f"h1aT{s}", tag=f"h1aT{s}")
                nc.sync.dma_start_transpose(
                    out=t[:], in_=h1a_loc[:, s * P:(s + 1) * P])
                h1aT.append(t)
            nc.gpsimd.collective_compute(
                "AllGather", mybir.AluOpType.bypass, replica_groups=rg,
                ins=[h1a_l8.opt()], outs=[h1a_sh.opt()])

            # -------- layer 1: papers (writes: src authors -> dst papers)
            conv("W1", xa_cmp, IN, [W["c1w_Wl"]], [W["c1w_Wr"]], [xpT],
                 None, None, bias_t.get("bias_p1"), h1p_l8, h1p_loc, None, 0)
            h1pT = []
            for s in range(2):
                t = pp.tile([P, P_PAD], f16, name=f"h1pT{s}", tag=f"h1pT{s}")
                nc.sync.dma_start_transpose(
                    out=t[:], in_=h1p_loc[:, s * P:(s + 1) * P])
                h1pT.append(t)

            # -------- layer 2: papers (gathers h1a from shared table)
            pool_p = psP.tile([1, H], f32, name="pool_p", tag="pool_p",
                              space="PSUM")
            pool_a = psP.tile([1, H], f32, name="pool_a", tag="pool_a",
                              space="PSUM")
            gW2 = conv("W2", h1a_sh, H, [W["c2w_Wl0"], W["c2w_Wl1"]],
                       [W["c2w_Wr0"], W["c2w_Wr1"]], h1pT, W["skipP_W"], xpT,
                       bias_t.get("bias_p2"), None, None, pool_p, 1)

            # AG(h1p): L2-papers does not consume it; pin it behind the last
            # L2-papers gather so the scheduler cannot hoist its inline wait
            # into the middle of the gather stream
            ccP = nc.gpsimd.collective_compute(
                "AllGather", mybir.AluOpType.bypass, replica_groups=rg,
                ins=[h1p_l8.opt()], outs=[h1p_sh.opt()])
            _add_dep_helper(ccP.ins, gW2[-1].ins,
                            reason="keep AG(h1p) after L2-papers gathers")

            # -------- layer 2: authors
            conv("B2", h1p_sh, H, [W["c2b_Wl0"], W["c2b_Wl1"]],
                 [W["c2b_Wr0"], W["c2b_Wr1"]], h1aT, W["skipA_W"], xaT,
                 bias_t.get("bias_a2"), None, None, pool_a, 2)

            pool_sb = wk.tile([1, 2 * H], f32, tag="poolout")
            nc.vector.tensor_copy(out=pool_sb[:, 0:H], in_=pool_a[:])
            nc.vector.tensor_copy(out=pool_sb[:, H:2 * H], in_=pool_p[:])
            nc.sync.dma_start(out=out_pool[:], in_=pool_sb[:])

            if debug:
                nc.sync.dma_start(out=dbg_h1a[:], in_=h1a_loc[:])
                nc.sync.dma_start(out=dbg_h1p[:], in_=h1p_loc[:])

    nc.compile()
    return nc


def kernel(**inputs):
    debug = bool(int(os.environ.get("GNN_DEBUG", "0")))
    trace = bool(int(os.environ.get("GNN_TRACE", "0")))
    rels, in_maps, bias_nz = _prep(inputs)
    nc = _build(rels, bias_nz, debug=debug)
    res = bass_utils.run_bass_kernel_spmd(
        nc, in_maps, core_ids=list(range(C)), trace=trace)
    kernel.last_results = res

    pools = np.stack([res.results[c]["out_pool"] for c in range(C)])
    sum_a = pools[:, 0, :H].astype(np.float64).sum(axis=0)
    sum_p = pools[:, 0, H:].astype(np.float64).sum(axis=0)
    pooled = np.concatenate([sum_a / NA, sum_p / NP_])[None, :]
    W1 = np.asarray(inputs["cls_W1"], np.float64)
    b1 = np.asarray(inputs["cls_b1"], np.float64)
    W2 = np.asarray(inputs["cls_W2"], np.float64)
    b2 = np.asarray(inputs["cls_b2"], np.float64)
    h = np.maximum(pooled @ W1.T + b1, 0.0)
    out = h @ W2.T + b2
    return out.astype(np.float32)
